# revision 20
# baseline (speedup 1.0000x reference)
"""Self-contained Trainium2 Bass kernel for the HQNN problem (v2).

Math: the 4-qubit circuit after angle embedding applies a fixed unitary whose
Heisenberg-evolved Z observables are sparse Pauli sums over {I,Y,Z}; each
hybrid layer reduces to tanh -> sin/cos -> a few elementwise products -> small
static matmuls (folded with the next Dense layer). Data-parallel over 8 cores.

v2: fp16 datapath. x is cast fp32->fp16 by a gpsimd DMA into DRAM scratch,
then DMA-transposed (XBAR) straight into SBUF in feature-major layout -- no
PE transposes or PSUM->SBUF copies on the input side. All matmuls are fp16
(1 cyc/row vs 4 for fp32). DVE/ACT passes are 1024 wide; partition shuffles
run on a uint32 view (halves 1x-mode shuffle cost).
"""
import sys
sys.path.insert(0, "/opt/trn_rl_repo")
import itertools
import contextlib
import numpy as np

import concourse.bass as bass
import concourse.bacc as bacc
import concourse.tile as tile
from concourse import mybir
from concourse.bass_utils import run_bass_kernel_spmd
from concourse.masks import make_identity

F32 = mybir.dt.float32
F16 = mybir.dt.float16
F32R = mybir.dt.float32r
U32 = mybir.dt.uint32
PI2 = float(np.pi / 2)
N_CORES = 8
B_TOTAL, D_IN = 524288, 16
B_CORE = B_TOTAL // N_CORES
N_SS = B_CORE // 16384  # supersteps of 16384 rows
CH = 4                  # chunks of 4096 rows per superstep

# ---------------- host-side math ----------------
_I2 = np.eye(2, dtype=complex)
_PY = np.array([[0, -1j], [1j, 0]])
_PZ = np.array([[1, 0], [0, -1]], dtype=complex)
SUPPORTS = [(0, 1, 3), (0, 2, 3), (1, 3), (0, 2)]


def _kron(ms):
    out = np.array([[1.0 + 0j]])
    for m in ms:
        out = np.kron(out, m)
    return out


def _op_on(w, m):
    return _kron([m if v == w else _I2 for v in range(4)])


def _layer_tensors(theta_l):
    U = np.eye(16, dtype=complex)
    for l in range(2):
        for w in range(4):
            c, s = np.cos(theta_l[l, w] / 2), np.sin(theta_l[l, w] / 2)
            U = _op_on(w, np.array([[c, -1j * s], [-1j * s, c]])) @ U
        for w in range(4):
            t = (w + 1) % 4
            C = np.zeros((16, 16), dtype=complex)
            for k in range(16):
                bits = [(k >> (3 - v)) & 1 for v in range(4)]
                if bits[w] == 1:
                    bits[t] ^= 1
                C[sum(b << (3 - v) for v, b in enumerate(bits)), k] = 1
            U = C @ U
    letters = {"I": _I2, "Y": _PY, "Z": _PZ}
    out = []
    for w, sup in enumerate(SUPPORTS):
        H = U.conj().T @ _op_on(w, _PZ) @ U
        T = np.zeros((2,) * len(sup))
        for s in itertools.product("IYZ", repeat=4):
            P = _kron([letters[c] for c in s])
            co = float(np.real(np.trace(P.conj().T @ H) / 16))
            if abs(co) < 1e-10:
                continue
            nz = tuple(v for v in range(4) if s[v] != "I")
            assert set(nz).issubset(set(sup)), f"support {s} w={w}"
            idx, sign = [], 1.0
            ok = True
            for v in sup:
                if s[v] == "I":
                    ok = False
                    break
                idx.append(0 if s[v] == "Y" else 1)
                if s[v] == "Y":
                    sign = -sign
            if not ok:
                assert abs(co) < 1e-10
                continue
            T[tuple(idx)] = sign * co
        out.append(T)
    return out  # C0, C1, B2, B3


def _blockdiag(blk, n):
    K, M = blk.shape
    out = np.zeros((K * n, M * n), dtype=np.float32)
    for i in range(n):
        out[i * K:(i + 1) * K, i * M:(i + 1) * M] = blk
    return out


WMAP = [3, 0, 1, 2, 3, 0, 1, 2]


def _to_bf16(a):
    return a.astype(np.float16)


def host_tensors(theta, W0, b0, W1, b1, W2, b2):
    t = {}
    for i in range(3):
        C0, C1, B2, B3 = _layer_tensors(np.asarray(theta[i], dtype=np.float64))
        A1 = np.zeros((8, 8), dtype=np.float32)
        for a in range(2):
            for c in range(2):
                gi = a * 2 + c
                A1[gi, 1] = C0[a, 0, c]
                A1[gi, 5] = C0[a, 1, c]
                A1[gi, 2] = C1[a, 0, c]
                A1[gi, 6] = C1[a, 1, c]
        A2 = np.zeros((8, 8), dtype=np.float32)
        for b in range(2):
            A2[1 + 4 * b, 3] = B2[b, 0]
            A2[1 + 4 * b, 7] = B2[b, 1]
            A2[2 + 4 * b, 0] = B3[0, b]
            A2[2 + 4 * b, 4] = B3[1, b]
        t[f"lA1_{i}"] = _blockdiag(A1, 16)
        t[f"lA2_{i}"] = _blockdiag(A2, 16)
    D0 = np.zeros((16, 8), dtype=np.float32)
    D0[:, 0:4] = W0
    D0[:, 4:8] = W0
    t["lD0"] = _blockdiag(D0, 8)
    for i, W in [(1, W1), (2, W2)]:
        D = np.zeros((8, 8), dtype=np.float32)
        for k in range(8):
            for j in range(4):
                D[k, j] = W[WMAP[k], j]
                D[k, j + 4] = W[WMAP[k], j]
        t[f"lD{i}"] = _blockdiag(D, 16)
    PO = np.zeros((8, 4), dtype=np.float32)
    for k in range(8):
        PO[k, WMAP[k]] = 1.0
    t["lPO"] = _blockdiag(PO, 16)
    consts = np.zeros((128, 4), dtype=np.float32)
    for i, b in enumerate((b0, b1, b2)):
        consts[:, i] = np.tile(np.tile(np.asarray(b, np.float32), 2), 16)
    consts[:, 3] = np.tile([0., 0., 0., 0., PI2, PI2, PI2, PI2], 16)
    t["consts"] = consts
    for name in list(t):
        if name not in ("consts", "lD0"):
            t[name] = _to_bf16(t[name])
    return t


# ---------------- device kernel ----------------
MASK_A = [0, 0, 4, 4, 0, 0, 0, 0]
MASK_B = [3, 7, 3, 7, 0, 0, 0, 0]
W16_NAMES = ["lD0", "lD1", "lD2", "lA1_0", "lA2_0", "lA1_1", "lA2_1",
             "lA1_2", "lA2_2", "lPO"]
W_NAMES = W16_NAMES + ["consts"]
W_COLS = {"lD0": 64, "lD1": 128, "lD2": 128, "lA1_0": 128, "lA2_0": 128,
          "lA1_1": 128, "lA2_1": 128, "lA1_2": 128, "lA2_2": 128,
          "lPO": 64, "consts": 4}


N_ST = 8  # steps of 8192 rows


def build_kernel(tc, x, out, wins):
    nc = tc.nc
    shufA = [8 * t_ + MASK_A[j] for t_ in range(4) for j in range(8)]
    shufB = [8 * t_ + MASK_B[j] for t_ in range(4) for j in range(8)]
    with contextlib.ExitStack() as ctx:
        wpool = ctx.enter_context(tc.tile_pool(name="w", bufs=1))
        slabp = ctx.enter_context(tc.tile_pool(name="slab", bufs=4))
        sxkp = ctx.enter_context(tc.tile_pool(name="sxk", bufs=2))
        work = ctx.enter_context(tc.tile_pool(name="work", bufs=4))
        outp = ctx.enter_context(tc.tile_pool(name="outp", bufs=2))
        ps_pre = ctx.enter_context(tc.tile_pool(name="ps_pre", bufs=4, space="PSUM"))
        ps_r1 = ctx.enter_context(tc.tile_pool(name="ps_r1", bufs=2, space="PSUM"))
        ps_ob = ctx.enter_context(tc.tile_pool(name="ps_ob", bufs=2, space="PSUM"))

        wt = {}
        for name in W16_NAMES:
            dt = F32 if name == "lD0" else F16
            wtile = wpool.tile([128, W_COLS[name]], dt, tag=name)
            nc.sync.dma_start(wtile[:], wins[name][:, :])
            wt[name] = wtile
        ctile = wpool.tile([128, 4], F32, tag="consts")
        nc.sync.dma_start(ctile[:], wins["consts"][:, :])
        lA1 = [wt["lA1_0"], wt["lA1_1"], wt["lA1_2"]]
        lA2 = [wt["lA2_0"], wt["lA2_1"], wt["lA2_2"]]
        lD = [None, wt["lD1"], wt["lD2"]]

        # block-swapped slab loads: 4 partition-block DMAs per (st, hh) land
        # the half in a layout where one DVE 32x32 block-transpose on
        # [128, 512] yields the full transpose (no PSUM, no copies, no cast).
        # row = ((st*4+s)*128 + (j*32+a))*16 + (hh*8 + i*2 + b1)
        xr = x.rearrange("(st s j a hh i b1) f -> st hh i a s j (b1 f)",
                         st=N_ST, s=4, j=4, a=32, hh=2, i=4, b1=2)
        ov = out.rearrange("(st s p r) w -> st s p (r w)", st=N_ST, s=4, p=128)

        for st in range(N_ST):
            sxk = sxkp.tile([128, 1024], F32, tag="sxk")
            for hh in range(2):
                slab = slabp.tile([128, 512], F32, tag="slab")
                for i in range(4):
                    nc.sync.dma_start(slab[32 * i:32 * i + 32, :], xr[st, hh, i])
                nc.vector.transpose(sxk[:, hh * 512:(hh + 1) * 512], slab[:])

            pre = ps_pre.tile([128, 512], F32, tag="pre", name="pre")
            nc.tensor.matmul(pre[0:64, :], wt["lD0"][:], sxk[:, 0:512],
                             start=True, stop=True)
            nc.tensor.matmul(pre[64:128, :], wt["lD0"][:], sxk[:, 512:1024],
                             start=True, stop=True)

            vin = None
            for li in range(3):
                if li > 0:
                    pre = ps_pre.tile([128, 512], F32, tag="pre", name="pre")
                    nc.tensor.matmul(pre[:], lD[li][:], vin[:], start=True, stop=True)
                h8 = work.tile([128, 512], F16, tag="h8")
                nc.scalar.activation(h8[:], pre[:],
                                     mybir.ActivationFunctionType.Tanh,
                                     bias=ctile[:, li:li + 1], scale=1.0)
                trig = work.tile([128, 512], F16, tag="trig")
                nc.scalar.activation(trig[:], h8[:],
                                     mybir.ActivationFunctionType.Sin,
                                     bias=ctile[:, 3:4], scale=1.0)
                ga = work.tile([128, 512], F16, tag="ga")
                gb = work.tile([128, 512], F16, tag="gb")
                nc.vector.stream_shuffle(ga[:].bitcast(U32), trig[:].bitcast(U32), shufA)
                nc.vector.stream_shuffle(gb[:].bitcast(U32), trig[:].bitcast(U32), shufB)
                g = work.tile([128, 512], F16, tag="g")
                nc.gpsimd.tensor_mul(g[:], ga[:], gb[:])
                r1 = ps_r1.tile([128, 512], F32, tag="r1", name="r1")
                nc.tensor.matmul(r1[:], lA1[li][:], g[:], start=True, stop=False)
                nc.tensor.matmul(r1[:], lA2[li][:], trig[:], start=False, stop=True)
                v = work.tile([128, 512], F16, tag="v")
                nc.vector.tensor_mul(v[:], trig[:], r1[:])
                vin = v

            # fused output projection+transpose: ob_chunk = v_chunk^T @ lPO
            ob = ps_ob.tile([128, 256], F32, tag="ob")
            for s in range(4):
                nc.tensor.matmul(ob[:, s * 64:(s + 1) * 64],
                                 vin[:, s * 128:(s + 1) * 128], wt["lPO"][:],
                                 start=True, stop=True)
            sob = outp.tile([128, 256], F32, tag="sob")
            nc.vector.tensor_copy(sob[:], ob[:])
            for s in range(4):
                nc.sync.dma_start(ov[st, s], sob[:, s * 64:(s + 1) * 64])


# Force Tanh/Sin into a single resident ACT table set (silu_and_others holds
# both) so the table-load pass doesn't thrash between per-func sets. Dict
# order/indices are preserved so act_func_set_id stays consistent.
from concourse import hw_specs as _hw_specs
import concourse.bacc as _bacc_mod
_orig_get_tables = _hw_specs.get_activation_tables

def _patched_get_tables(arch):
    tabs = _orig_get_tables(arch)
    out = {}
    for name, s in tabs.items():
        s2 = set(s)
        if name != "silu_and_others":
            s2.discard(mybir.ActivationFunctionType.Tanh)
            s2.discard(mybir.ActivationFunctionType.Sin)
        out[name] = s2
    return out

_hw_specs.get_activation_tables = _patched_get_tables
for _mod in (_bacc_mod,):
    if hasattr(_mod, "get_activation_tables"):
        _mod.get_activation_tables = _patched_get_tables


_CACHE = {}


def _get_compiled():
    if "nc" in _CACHE:
        return _CACHE["nc"], _CACHE["tiles"]
    nc = bacc.Bacc("TRN2", target_bir_lowering=False, debug=False,
                   num_devices=N_CORES)
    x_ap = nc.dram_tensor("x", [B_CORE, D_IN], F32, kind="ExternalInput").ap()
    out_ap = nc.dram_tensor("out", [B_CORE, 4], F32, kind="ExternalOutput").ap()
    wins = {}
    for name in W16_NAMES:
        dt = F32 if name == "lD0" else F16
        wins[name] = nc.dram_tensor(name, [128, W_COLS[name]], dt,
                                    kind="ExternalInput").ap()
    wins["consts"] = nc.dram_tensor("consts", [128, 4], F32,
                                    kind="ExternalInput").ap()
    with tile.TileContext(nc) as tc:
        build_kernel(tc, x_ap, out_ap, wins)
    nc.compile()
    _CACHE["nc"] = nc
    _CACHE["tiles"] = None
    return nc, None


def kernel(x, theta, W0, b0, W1, b1, W2, b2):
    x = np.ascontiguousarray(np.asarray(x, dtype=np.float32))
    wt = host_tensors(np.asarray(theta), np.asarray(W0), np.asarray(b0),
                      np.asarray(W1), np.asarray(b1), np.asarray(W2),
                      np.asarray(b2))
    nc, _ = _get_compiled()
    in_maps = []
    for c in range(N_CORES):
        m = {"x": np.ascontiguousarray(x[c * B_CORE:(c + 1) * B_CORE])}
        for name in W_NAMES:
            m[name] = wt[name]
        in_maps.append(m)
    res = run_bass_kernel_spmd(nc, in_maps, core_ids=list(range(N_CORES)))
    outs = [res.results[c]["out"] for c in range(N_CORES)]
    return np.concatenate(outs, axis=0).astype(np.float32)


# revision 22
# speedup vs baseline: 1.2113x; 1.2113x over previous
"""Self-contained Trainium2 Bass kernel for the HQNN problem (v2).

Math: the 4-qubit circuit after angle embedding applies a fixed unitary whose
Heisenberg-evolved Z observables are sparse Pauli sums over {I,Y,Z}; each
hybrid layer reduces to tanh -> sin/cos -> a few elementwise products -> small
static matmuls (folded with the next Dense layer). Data-parallel over 8 cores.

v2: fp16 datapath. x is cast fp32->fp16 by a gpsimd DMA into DRAM scratch,
then DMA-transposed (XBAR) straight into SBUF in feature-major layout -- no
PE transposes or PSUM->SBUF copies on the input side. All matmuls are fp16
(1 cyc/row vs 4 for fp32). DVE/ACT passes are 1024 wide; partition shuffles
run on a uint32 view (halves 1x-mode shuffle cost).
"""
import sys
sys.path.insert(0, "/opt/trn_rl_repo")
import itertools
import contextlib
import numpy as np

import concourse.bass as bass
import concourse.bacc as bacc
import concourse.tile as tile
from concourse import mybir
from concourse.bass_utils import run_bass_kernel_spmd
from concourse.masks import make_identity

F32 = mybir.dt.float32
F16 = mybir.dt.float16
F32R = mybir.dt.float32r
U32 = mybir.dt.uint32
PI2 = float(np.pi / 2)
N_CORES = 8
B_TOTAL, D_IN = 524288, 16
B_CORE = B_TOTAL // N_CORES
N_SS = B_CORE // 16384  # supersteps of 16384 rows
CH = 4                  # chunks of 4096 rows per superstep

# ---------------- host-side math ----------------
_I2 = np.eye(2, dtype=complex)
_PY = np.array([[0, -1j], [1j, 0]])
_PZ = np.array([[1, 0], [0, -1]], dtype=complex)
SUPPORTS = [(0, 1, 3), (0, 2, 3), (1, 3), (0, 2)]


def _kron(ms):
    out = np.array([[1.0 + 0j]])
    for m in ms:
        out = np.kron(out, m)
    return out


def _op_on(w, m):
    return _kron([m if v == w else _I2 for v in range(4)])


def _layer_tensors(theta_l):
    U = np.eye(16, dtype=complex)
    for l in range(2):
        for w in range(4):
            c, s = np.cos(theta_l[l, w] / 2), np.sin(theta_l[l, w] / 2)
            U = _op_on(w, np.array([[c, -1j * s], [-1j * s, c]])) @ U
        for w in range(4):
            t = (w + 1) % 4
            C = np.zeros((16, 16), dtype=complex)
            for k in range(16):
                bits = [(k >> (3 - v)) & 1 for v in range(4)]
                if bits[w] == 1:
                    bits[t] ^= 1
                C[sum(b << (3 - v) for v, b in enumerate(bits)), k] = 1
            U = C @ U
    letters = {"I": _I2, "Y": _PY, "Z": _PZ}
    out = []
    for w, sup in enumerate(SUPPORTS):
        H = U.conj().T @ _op_on(w, _PZ) @ U
        T = np.zeros((2,) * len(sup))
        for s in itertools.product("IYZ", repeat=4):
            P = _kron([letters[c] for c in s])
            co = float(np.real(np.trace(P.conj().T @ H) / 16))
            if abs(co) < 1e-10:
                continue
            nz = tuple(v for v in range(4) if s[v] != "I")
            assert set(nz).issubset(set(sup)), f"support {s} w={w}"
            idx, sign = [], 1.0
            ok = True
            for v in sup:
                if s[v] == "I":
                    ok = False
                    break
                idx.append(0 if s[v] == "Y" else 1)
                if s[v] == "Y":
                    sign = -sign
            if not ok:
                assert abs(co) < 1e-10
                continue
            T[tuple(idx)] = sign * co
        out.append(T)
    return out  # C0, C1, B2, B3


def _blockdiag(blk, n):
    K, M = blk.shape
    out = np.zeros((K * n, M * n), dtype=np.float32)
    for i in range(n):
        out[i * K:(i + 1) * K, i * M:(i + 1) * M] = blk
    return out


WMAP = [3, 0, 1, 2, 3, 0, 1, 2]


def _to_bf16(a):
    return a.astype(np.float16)


def host_tensors(theta, W0, b0, W1, b1, W2, b2):
    t = {}
    for i in range(3):
        C0, C1, B2, B3 = _layer_tensors(np.asarray(theta[i], dtype=np.float64))
        A1 = np.zeros((8, 8), dtype=np.float32)
        for a in range(2):
            for c in range(2):
                gi = a * 2 + c
                A1[gi, 1] = C0[a, 0, c]
                A1[gi, 5] = C0[a, 1, c]
                A1[gi, 2] = C1[a, 0, c]
                A1[gi, 6] = C1[a, 1, c]
        A2 = np.zeros((8, 8), dtype=np.float32)
        for b in range(2):
            A2[1 + 4 * b, 3] = B2[b, 0]
            A2[1 + 4 * b, 7] = B2[b, 1]
            A2[2 + 4 * b, 0] = B3[0, b]
            A2[2 + 4 * b, 4] = B3[1, b]
        t[f"lA1_{i}"] = _blockdiag(A1, 16)
        t[f"lA2_{i}"] = _blockdiag(A2, 16)
    D0 = np.zeros((16, 8), dtype=np.float32)
    D0[:, 0:4] = W0
    D0[:, 4:8] = W0
    t["lD0"] = _blockdiag(D0, 8)
    for i, W in [(1, W1), (2, W2)]:
        D = np.zeros((8, 8), dtype=np.float32)
        for k in range(8):
            for j in range(4):
                D[k, j] = W[WMAP[k], j]
                D[k, j + 4] = W[WMAP[k], j]
        t[f"lD{i}"] = _blockdiag(D, 16)
    PO = np.zeros((8, 4), dtype=np.float32)
    for k in range(8):
        PO[k, WMAP[k]] = 1.0
    t["lPO"] = _blockdiag(PO, 16)
    consts = np.zeros((128, 4), dtype=np.float32)
    for i, b in enumerate((b0, b1, b2)):
        consts[:, i] = np.tile(np.tile(np.asarray(b, np.float32), 2), 16)
    consts[:, 3] = np.tile([0., 0., 0., 0., PI2, PI2, PI2, PI2], 16)
    t["consts"] = consts
    for name in list(t):
        if name != "consts":
            t[name] = _to_bf16(t[name])
    return t


# ---------------- device kernel ----------------
MASK_A = [0, 0, 4, 4, 0, 0, 0, 0]
MASK_B = [3, 7, 3, 7, 0, 0, 0, 0]
W16_NAMES = ["lD0", "lD1", "lD2", "lA1_0", "lA2_0", "lA1_1", "lA2_1",
             "lA1_2", "lA2_2", "lPO"]
W_NAMES = W16_NAMES + ["consts"]
W_COLS = {"lD0": 64, "lD1": 128, "lD2": 128, "lA1_0": 128, "lA2_0": 128,
          "lA1_1": 128, "lA2_1": 128, "lA1_2": 128, "lA2_2": 128,
          "lPO": 64, "consts": 4}


N_ST = 8  # steps of 8192 rows


def build_kernel(tc, x, out, wins):
    nc = tc.nc
    shufA = [8 * t_ + MASK_A[j] for t_ in range(4) for j in range(8)]
    shufB = [8 * t_ + MASK_B[j] for t_ in range(4) for j in range(8)]
    with contextlib.ExitStack() as ctx:
        wpool = ctx.enter_context(tc.tile_pool(name="w", bufs=1))
        dram = ctx.enter_context(tc.tile_pool(name="dram", bufs=8, space="DRAM"))
        slabp = ctx.enter_context(tc.tile_pool(name="slab", bufs=8))
        sxkp = ctx.enter_context(tc.tile_pool(name="sxk", bufs=2))
        work = ctx.enter_context(tc.tile_pool(name="work", bufs=4))
        outp = ctx.enter_context(tc.tile_pool(name="outp", bufs=2))
        ps_xk = ctx.enter_context(tc.tile_pool(name="ps_xk", bufs=2, space="PSUM"))
        ps_pre = ctx.enter_context(tc.tile_pool(name="ps_pre", bufs=2, space="PSUM"))
        ps_r1 = ctx.enter_context(tc.tile_pool(name="ps_r1", bufs=2, space="PSUM"))
        ps_ob = ctx.enter_context(tc.tile_pool(name="ps_ob", bufs=2, space="PSUM"))

        wt = {}
        for name in W16_NAMES:
            wtile = wpool.tile([128, W_COLS[name]], F16, tag=name)
            nc.sync.dma_start(wtile[:], wins[name][:, :])
            wt[name] = wtile
        ctile = wpool.tile([128, 4], F32, tag="consts")
        nc.sync.dma_start(ctile[:], wins["consts"][:, :])
        identf = wpool.tile([128, 128], F16, tag="identf")
        make_identity(nc, identf)
        lA1 = [wt["lA1_0"], wt["lA1_1"], wt["lA1_2"]]
        lA2 = [wt["lA2_0"], wt["lA2_1"], wt["lA2_2"]]
        lD = [None, wt["lD1"], wt["lD2"]]

        xin = x.rearrange("(st k) f -> st k f", st=N_ST)
        ov = out.rearrange("(st s p r) w -> st s p (r w)", st=N_ST, s=4, p=128)

        # upfront fp32->fp16 casts into DRAM scratch, one per step; SWDGE
        # DRAM->DRAM runs at high rate on big contiguous descriptors
        x16s = []
        for st in range(N_ST):
            x16 = dram.tile([8192, 16], F16, tag="x16", name="x16")
            nc.gpsimd.dma_start(x16[:], xin[st])
            x16s.append(x16)

        for st in range(N_ST):
            x16v = x16s[st][:].rearrange("(s p r) f -> s p (r f)", s=4, p=128)
            xkps = ps_xk.tile([128, 1024], F16, tag="xkps")
            for s in range(4):
                slab = slabp.tile([128, 256], F16, tag="slab")
                nc.sync.dma_start(slab[:], x16v[s])
                nc.tensor.transpose(xkps[:, 0 * 512 + s * 128:0 * 512 + s * 128 + 128],
                                    slab[:, 0:128], identf[:])
                nc.tensor.transpose(xkps[:, 1 * 512 + s * 128:1 * 512 + s * 128 + 128],
                                    slab[:, 128:256], identf[:])
            sxk = sxkp.tile([128, 1024], F16, tag="sxk")
            nc.vector.tensor_copy(sxk[:, 0:512], xkps[:, 0:512])
            nc.scalar.copy(sxk[:, 512:1024], xkps[:, 512:1024])

            pre = ps_pre.tile([128, 512], F32, tag="pre", name="pre")
            nc.tensor.matmul(pre[0:64, :], wt["lD0"][:], sxk[:, 0:512],
                             start=True, stop=True)
            nc.tensor.matmul(pre[64:128, :], wt["lD0"][:], sxk[:, 512:1024],
                             start=True, stop=True)

            vin = None
            for li in range(3):
                if li > 0:
                    pre = ps_pre.tile([128, 512], F32, tag="pre", name="pre")
                    nc.tensor.matmul(pre[:], lD[li][:], vin[:], start=True, stop=True)
                h8 = work.tile([128, 512], F16, tag="h8")
                nc.scalar.activation(h8[:], pre[:],
                                     mybir.ActivationFunctionType.Tanh,
                                     bias=ctile[:, li:li + 1], scale=1.0)
                trig = work.tile([128, 512], F16, tag="trig")
                nc.scalar.activation(trig[:], h8[:],
                                     mybir.ActivationFunctionType.Sin,
                                     bias=ctile[:, 3:4], scale=1.0)
                ga = work.tile([128, 512], F16, tag="ga")
                gb = work.tile([128, 512], F16, tag="gb")
                nc.vector.stream_shuffle(ga[:].bitcast(U32), trig[:].bitcast(U32), shufA)
                nc.vector.stream_shuffle(gb[:].bitcast(U32), trig[:].bitcast(U32), shufB)
                g = work.tile([128, 512], F16, tag="g")
                nc.gpsimd.tensor_mul(g[:], ga[:], gb[:])
                r1 = ps_r1.tile([128, 512], F32, tag="r1", name="r1")
                nc.tensor.matmul(r1[:], lA1[li][:], g[:], start=True, stop=False)
                nc.tensor.matmul(r1[:], lA2[li][:], trig[:], start=False, stop=True)
                v = work.tile([128, 512], F16, tag="v")
                nc.vector.tensor_mul(v[:], trig[:], r1[:])
                vin = v

            # fused output projection+transpose: ob_chunk = v_chunk^T @ lPO
            ob = ps_ob.tile([128, 256], F32, tag="ob")
            for s in range(4):
                nc.tensor.matmul(ob[:, s * 64:(s + 1) * 64],
                                 vin[:, s * 128:(s + 1) * 128], wt["lPO"][:],
                                 start=True, stop=True)
            sob = outp.tile([128, 256], F32, tag="sob")
            nc.scalar.copy(sob[:], ob[:])
            for s in range(4):
                nc.sync.dma_start(ov[st, s], sob[:, s * 64:(s + 1) * 64])


# Force Tanh/Sin into a single resident ACT table set (silu_and_others holds
# both) so the table-load pass doesn't thrash between per-func sets. Dict
# order/indices are preserved so act_func_set_id stays consistent.
from concourse import hw_specs as _hw_specs
import concourse.bacc as _bacc_mod
_orig_get_tables = _hw_specs.get_activation_tables

def _patched_get_tables(arch):
    tabs = _orig_get_tables(arch)
    out = {}
    for name, s in tabs.items():
        s2 = set(s)
        if name != "silu_and_others":
            s2.discard(mybir.ActivationFunctionType.Tanh)
            s2.discard(mybir.ActivationFunctionType.Sin)
        out[name] = s2
    return out

_hw_specs.get_activation_tables = _patched_get_tables
for _mod in (_bacc_mod,):
    if hasattr(_mod, "get_activation_tables"):
        _mod.get_activation_tables = _patched_get_tables


_CACHE = {}


def _get_compiled():
    if "nc" in _CACHE:
        return _CACHE["nc"], _CACHE["tiles"]
    nc = bacc.Bacc("TRN2", target_bir_lowering=False, debug=False,
                   num_devices=N_CORES)
    x_ap = nc.dram_tensor("x", [B_CORE, D_IN], F32, kind="ExternalInput").ap()
    out_ap = nc.dram_tensor("out", [B_CORE, 4], F32, kind="ExternalOutput").ap()
    wins = {}
    for name in W16_NAMES:
        wins[name] = nc.dram_tensor(name, [128, W_COLS[name]], F16,
                                    kind="ExternalInput").ap()
    wins["consts"] = nc.dram_tensor("consts", [128, 4], F32,
                                    kind="ExternalInput").ap()
    with tile.TileContext(nc) as tc:
        build_kernel(tc, x_ap, out_ap, wins)
    nc.compile()
    _CACHE["nc"] = nc
    _CACHE["tiles"] = None
    return nc, None


def kernel(x, theta, W0, b0, W1, b1, W2, b2):
    x = np.ascontiguousarray(np.asarray(x, dtype=np.float32))
    wt = host_tensors(np.asarray(theta), np.asarray(W0), np.asarray(b0),
                      np.asarray(W1), np.asarray(b1), np.asarray(W2),
                      np.asarray(b2))
    nc, _ = _get_compiled()
    in_maps = []
    for c in range(N_CORES):
        m = {"x": np.ascontiguousarray(x[c * B_CORE:(c + 1) * B_CORE])}
        for name in W_NAMES:
            m[name] = wt[name]
        in_maps.append(m)
    res = run_bass_kernel_spmd(nc, in_maps, core_ids=list(range(N_CORES)))
    outs = [res.results[c]["out"] for c in range(N_CORES)]
    return np.concatenate(outs, axis=0).astype(np.float32)


# revision 23
# speedup vs baseline: 1.8614x; 1.5368x over previous
"""Self-contained Trainium2 Bass kernel for the HQNN problem (v2).

Math: the 4-qubit circuit after angle embedding applies a fixed unitary whose
Heisenberg-evolved Z observables are sparse Pauli sums over {I,Y,Z}; each
hybrid layer reduces to tanh -> sin/cos -> a few elementwise products -> small
static matmuls (folded with the next Dense layer). Data-parallel over 8 cores.

v2: fp16 datapath. x is cast fp32->fp16 by a gpsimd DMA into DRAM scratch,
then DMA-transposed (XBAR) straight into SBUF in feature-major layout -- no
PE transposes or PSUM->SBUF copies on the input side. All matmuls are fp16
(1 cyc/row vs 4 for fp32). DVE/ACT passes are 1024 wide; partition shuffles
run on a uint32 view (halves 1x-mode shuffle cost).
"""
import sys
sys.path.insert(0, "/opt/trn_rl_repo")
import itertools
import contextlib
import numpy as np

import concourse.bass as bass
import concourse.bacc as bacc
import concourse.tile as tile
from concourse import mybir
from concourse.bass_utils import run_bass_kernel_spmd
from concourse.masks import make_identity

F32 = mybir.dt.float32
F16 = mybir.dt.float16
F32R = mybir.dt.float32r
U32 = mybir.dt.uint32
PI2 = float(np.pi / 2)
N_CORES = 8
B_TOTAL, D_IN = 524288, 16
B_CORE = B_TOTAL // N_CORES
N_SS = B_CORE // 16384  # supersteps of 16384 rows
CH = 4                  # chunks of 4096 rows per superstep

# ---------------- host-side math ----------------
_I2 = np.eye(2, dtype=complex)
_PY = np.array([[0, -1j], [1j, 0]])
_PZ = np.array([[1, 0], [0, -1]], dtype=complex)
SUPPORTS = [(0, 1, 3), (0, 2, 3), (1, 3), (0, 2)]


def _kron(ms):
    out = np.array([[1.0 + 0j]])
    for m in ms:
        out = np.kron(out, m)
    return out


def _op_on(w, m):
    return _kron([m if v == w else _I2 for v in range(4)])


def _layer_tensors(theta_l):
    U = np.eye(16, dtype=complex)
    for l in range(2):
        for w in range(4):
            c, s = np.cos(theta_l[l, w] / 2), np.sin(theta_l[l, w] / 2)
            U = _op_on(w, np.array([[c, -1j * s], [-1j * s, c]])) @ U
        for w in range(4):
            t = (w + 1) % 4
            C = np.zeros((16, 16), dtype=complex)
            for k in range(16):
                bits = [(k >> (3 - v)) & 1 for v in range(4)]
                if bits[w] == 1:
                    bits[t] ^= 1
                C[sum(b << (3 - v) for v, b in enumerate(bits)), k] = 1
            U = C @ U
    letters = {"I": _I2, "Y": _PY, "Z": _PZ}
    out = []
    for w, sup in enumerate(SUPPORTS):
        H = U.conj().T @ _op_on(w, _PZ) @ U
        T = np.zeros((2,) * len(sup))
        for s in itertools.product("IYZ", repeat=4):
            P = _kron([letters[c] for c in s])
            co = float(np.real(np.trace(P.conj().T @ H) / 16))
            if abs(co) < 1e-10:
                continue
            nz = tuple(v for v in range(4) if s[v] != "I")
            assert set(nz).issubset(set(sup)), f"support {s} w={w}"
            idx, sign = [], 1.0
            ok = True
            for v in sup:
                if s[v] == "I":
                    ok = False
                    break
                idx.append(0 if s[v] == "Y" else 1)
                if s[v] == "Y":
                    sign = -sign
            if not ok:
                assert abs(co) < 1e-10
                continue
            T[tuple(idx)] = sign * co
        out.append(T)
    return out  # C0, C1, B2, B3


def _blockdiag(blk, n):
    K, M = blk.shape
    out = np.zeros((K * n, M * n), dtype=np.float32)
    for i in range(n):
        out[i * K:(i + 1) * K, i * M:(i + 1) * M] = blk
    return out


WMAP = [3, 0, 1, 2, 3, 0, 1, 2]


def _to_bf16(a):
    return a.astype(np.float16)


def host_tensors(theta, W0, b0, W1, b1, W2, b2):
    t = {}
    for i in range(3):
        C0, C1, B2, B3 = _layer_tensors(np.asarray(theta[i], dtype=np.float64))
        A1 = np.zeros((8, 8), dtype=np.float32)
        for a in range(2):
            for c in range(2):
                gi = a * 2 + c
                A1[gi, 1] = C0[a, 0, c]
                A1[gi, 5] = C0[a, 1, c]
                A1[gi, 2] = C1[a, 0, c]
                A1[gi, 6] = C1[a, 1, c]
        A2 = np.zeros((8, 8), dtype=np.float32)
        for b in range(2):
            A2[1 + 4 * b, 3] = B2[b, 0]
            A2[1 + 4 * b, 7] = B2[b, 1]
            A2[2 + 4 * b, 0] = B3[0, b]
            A2[2 + 4 * b, 4] = B3[1, b]
        t[f"lA1_{i}"] = _blockdiag(A1, 16)
        t[f"lA2_{i}"] = _blockdiag(A2, 16)
    D0 = np.zeros((16, 8), dtype=np.float32)
    D0[:, 0:4] = W0
    D0[:, 4:8] = W0
    t["lD0"] = _blockdiag(D0, 8)
    for i, W in [(1, W1), (2, W2)]:
        D = np.zeros((8, 8), dtype=np.float32)
        for k in range(8):
            for j in range(4):
                D[k, j] = W[WMAP[k], j]
                D[k, j + 4] = W[WMAP[k], j]
        t[f"lD{i}"] = _blockdiag(D, 16)
    PO = np.zeros((8, 4), dtype=np.float32)
    for k in range(8):
        PO[k, WMAP[k]] = 1.0
    t["lPO"] = _blockdiag(PO, 16)
    consts = np.zeros((128, 4), dtype=np.float32)
    for i, b in enumerate((b0, b1, b2)):
        consts[:, i] = np.tile(np.tile(np.asarray(b, np.float32), 2), 16)
    consts[:, 3] = np.tile([0., 0., 0., 0., PI2, PI2, PI2, PI2], 16)
    t["consts"] = consts
    for name in list(t):
        if name != "consts":
            t[name] = _to_bf16(t[name])
    return t


# ---------------- device kernel ----------------
MASK_A = [0, 0, 4, 4, 0, 0, 0, 0]
MASK_B = [3, 7, 3, 7, 0, 0, 0, 0]
W16_NAMES = ["lD0", "lD1", "lD2", "lA1_0", "lA2_0", "lA1_1", "lA2_1",
             "lA1_2", "lA2_2", "lPO"]
W_NAMES = W16_NAMES + ["consts"]
W_COLS = {"lD0": 64, "lD1": 128, "lD2": 128, "lA1_0": 128, "lA2_0": 128,
          "lA1_1": 128, "lA2_1": 128, "lA1_2": 128, "lA2_2": 128,
          "lPO": 64, "consts": 4}


N_ST = 8  # steps of 8192 rows


def build_kernel(tc, x, out, wins):
    nc = tc.nc
    shufA = [8 * t_ + MASK_A[j] for t_ in range(4) for j in range(8)]
    shufB = [8 * t_ + MASK_B[j] for t_ in range(4) for j in range(8)]
    with contextlib.ExitStack() as ctx:
        wpool = ctx.enter_context(tc.tile_pool(name="w", bufs=1))
        dram = ctx.enter_context(tc.tile_pool(name="dram", bufs=8, space="DRAM"))
        slabp = ctx.enter_context(tc.tile_pool(name="slab", bufs=8))
        sxkp = ctx.enter_context(tc.tile_pool(name="sxk", bufs=8))
        hp = ctx.enter_context(tc.tile_pool(name="hp", bufs=4))
        tp = ctx.enter_context(tc.tile_pool(name="tp", bufs=4))
        gp = ctx.enter_context(tc.tile_pool(name="gp", bufs=4))
        vp = ctx.enter_context(tc.tile_pool(name="vp", bufs=10))
        outp = ctx.enter_context(tc.tile_pool(name="outp", bufs=3))
        ps_xk = ctx.enter_context(tc.tile_pool(name="ps_xk", bufs=2, space="PSUM"))
        ps_pre = ctx.enter_context(tc.tile_pool(name="ps_pre", bufs=3, space="PSUM"))
        ps_r1 = ctx.enter_context(tc.tile_pool(name="ps_r1", bufs=2, space="PSUM"))
        ps_ob = ctx.enter_context(tc.tile_pool(name="ps_ob", bufs=1, space="PSUM"))

        wt = {}
        for name in W16_NAMES:
            wtile = wpool.tile([128, W_COLS[name]], F16, tag=name)
            nc.sync.dma_start(wtile[:], wins[name][:, :])
            wt[name] = wtile
        ctile = wpool.tile([128, 4], F32, tag="consts")
        nc.sync.dma_start(ctile[:], wins["consts"][:, :])
        identf = wpool.tile([128, 128], F16, tag="identf")
        make_identity(nc, identf)
        lA1 = [wt["lA1_0"], wt["lA1_1"], wt["lA1_2"]]
        lA2 = [wt["lA2_0"], wt["lA2_1"], wt["lA2_2"]]
        lD = [None, wt["lD1"], wt["lD2"]]

        xin = x.rearrange("(st k) f -> st k f", st=N_ST)
        ov = out.rearrange("(st s p r) w -> st s p (r w)", st=N_ST, s=4, p=128)

        # upfront fp32->fp16 casts into DRAM scratch (SWDGE, big descriptors)
        x16s = []
        for st in range(N_ST):
            x16 = dram.tile([8192, 16], F16, tag="x16", name="x16")
            nc.gpsimd.dma_start(x16[:], xin[st])
            x16s.append(x16)

        # ingestion for all steps: slabs -> PE transposes -> SBUF copies.
        # stage-major emission keeps each engine's in-order stream free of
        # cross-step head-of-line blocking.
        sxks = []
        for st in range(N_ST):
            x16v = x16s[st][:].rearrange("(s p r) f -> s p (r f)", s=4, p=128)
            xkps = ps_xk.tile([128, 1024], F16, tag="xkps", name="xkps")
            for s in range(4):
                slab = slabp.tile([128, 256], F16, tag="slab", name="slab")
                nc.sync.dma_start(slab[:], x16v[s])
                nc.tensor.transpose(xkps[:, s * 128:s * 128 + 128],
                                    slab[:, 0:128], identf[:])
                nc.tensor.transpose(xkps[:, 512 + s * 128:512 + s * 128 + 128],
                                    slab[:, 128:256], identf[:])
            sxk = sxkp.tile([128, 1024], F16, tag="sxk", name="sxk")
            nc.vector.tensor_copy(sxk[:, 0:512], xkps[:, 0:512])
            nc.vector.tensor_copy(sxk[:, 512:1024], xkps[:, 512:1024])
            sxks.append(sxk)

        vins = [None] * N_ST
        for li in range(3):
            pres, trigs, gs, r1s = [], [], [], []
            for st in range(N_ST):
                pre = ps_pre.tile([128, 512], F32, tag="pre", name="pre")
                if li == 0:
                    nc.tensor.matmul(pre[0:64, :], wt["lD0"][:],
                                     sxks[st][:, 0:512], start=True, stop=True)
                    nc.tensor.matmul(pre[64:128, :], wt["lD0"][:],
                                     sxks[st][:, 512:1024], start=True, stop=True)
                else:
                    nc.tensor.matmul(pre[:], lD[li][:], vins[st][:],
                                     start=True, stop=True)
                pres.append(pre)
            for st in range(N_ST):
                h8 = hp.tile([128, 512], F16, tag="h8", name="h8")
                nc.scalar.activation(h8[:], pres[st][:],
                                     mybir.ActivationFunctionType.Tanh,
                                     bias=ctile[:, li:li + 1], scale=1.0)
                trig = tp.tile([128, 512], F16, tag="trig", name="trig")
                nc.scalar.activation(trig[:], h8[:],
                                     mybir.ActivationFunctionType.Sin,
                                     bias=ctile[:, 3:4], scale=1.0)
                trigs.append(trig)
            for st in range(N_ST):
                ga = gp.tile([128, 512], F16, tag="ga", name="ga")
                gb = gp.tile([128, 512], F16, tag="gb", name="gb")
                nc.vector.stream_shuffle(ga[:].bitcast(U32),
                                         trigs[st][:].bitcast(U32), shufA)
                nc.vector.stream_shuffle(gb[:].bitcast(U32),
                                         trigs[st][:].bitcast(U32), shufB)
                g = gp.tile([128, 512], F16, tag="g", name="g")
                nc.gpsimd.tensor_mul(g[:], ga[:], gb[:])
                gs.append(g)
            # A-matmuls and vmuls, skewed so DVE's in-order stream never
            # stalls the PE accumulations
            def emit_A(st):
                r1 = ps_r1.tile([128, 512], F32, tag="r1", name="r1")
                nc.tensor.matmul(r1[:], lA1[li][:], gs[st][:], start=True, stop=False)
                nc.tensor.matmul(r1[:], lA2[li][:], trigs[st][:], start=False, stop=True)
                r1s.append(r1)

            def emit_v(st):
                v = vp.tile([128, 512], F16, tag="v", name="v")
                nc.vector.tensor_mul(v[:], trigs[st][:], r1s[st][:])
                vins[st] = v

            for st in range(N_ST):
                emit_A(st)
                if st >= 1:
                    emit_v(st - 1)
            emit_v(N_ST - 1)

        for st in range(N_ST):
            ob = ps_ob.tile([128, 256], F32, tag="ob", name="ob")
            for s in range(4):
                nc.tensor.matmul(ob[:, s * 64:(s + 1) * 64],
                                 vins[st][:, s * 128:(s + 1) * 128], wt["lPO"][:],
                                 start=True, stop=True)
            sob = outp.tile([128, 256], F32, tag="sob", name="sob")
            nc.scalar.copy(sob[:], ob[:])
            for s in range(4):
                nc.sync.dma_start(ov[st, s], sob[:, s * 64:(s + 1) * 64])


# Force Tanh/Sin into a single resident ACT table set (silu_and_others holds
# both) so the table-load pass doesn't thrash between per-func sets. Dict
# order/indices are preserved so act_func_set_id stays consistent.
from concourse import hw_specs as _hw_specs
import concourse.bacc as _bacc_mod
_orig_get_tables = _hw_specs.get_activation_tables

def _patched_get_tables(arch):
    tabs = _orig_get_tables(arch)
    out = {}
    for name, s in tabs.items():
        s2 = set(s)
        if name != "silu_and_others":
            s2.discard(mybir.ActivationFunctionType.Tanh)
            s2.discard(mybir.ActivationFunctionType.Sin)
        out[name] = s2
    return out

_hw_specs.get_activation_tables = _patched_get_tables
for _mod in (_bacc_mod,):
    if hasattr(_mod, "get_activation_tables"):
        _mod.get_activation_tables = _patched_get_tables


_CACHE = {}


def _get_compiled():
    if "nc" in _CACHE:
        return _CACHE["nc"], _CACHE["tiles"]
    nc = bacc.Bacc("TRN2", target_bir_lowering=False, debug=False,
                   num_devices=N_CORES)
    x_ap = nc.dram_tensor("x", [B_CORE, D_IN], F32, kind="ExternalInput").ap()
    out_ap = nc.dram_tensor("out", [B_CORE, 4], F32, kind="ExternalOutput").ap()
    wins = {}
    for name in W16_NAMES:
        wins[name] = nc.dram_tensor(name, [128, W_COLS[name]], F16,
                                    kind="ExternalInput").ap()
    wins["consts"] = nc.dram_tensor("consts", [128, 4], F32,
                                    kind="ExternalInput").ap()
    with tile.TileContext(nc) as tc:
        build_kernel(tc, x_ap, out_ap, wins)
    nc.compile()
    _CACHE["nc"] = nc
    _CACHE["tiles"] = None
    return nc, None


def kernel(x, theta, W0, b0, W1, b1, W2, b2):
    x = np.ascontiguousarray(np.asarray(x, dtype=np.float32))
    wt = host_tensors(np.asarray(theta), np.asarray(W0), np.asarray(b0),
                      np.asarray(W1), np.asarray(b1), np.asarray(W2),
                      np.asarray(b2))
    nc, _ = _get_compiled()
    in_maps = []
    for c in range(N_CORES):
        m = {"x": np.ascontiguousarray(x[c * B_CORE:(c + 1) * B_CORE])}
        for name in W_NAMES:
            m[name] = wt[name]
        in_maps.append(m)
    res = run_bass_kernel_spmd(nc, in_maps, core_ids=list(range(N_CORES)))
    outs = [res.results[c]["out"] for c in range(N_CORES)]
    return np.concatenate(outs, axis=0).astype(np.float32)


# revision 24
# speedup vs baseline: 2.1064x; 1.1316x over previous
"""Self-contained Trainium2 Bass kernel for the HQNN problem (v2).

Math: the 4-qubit circuit after angle embedding applies a fixed unitary whose
Heisenberg-evolved Z observables are sparse Pauli sums over {I,Y,Z}; each
hybrid layer reduces to tanh -> sin/cos -> a few elementwise products -> small
static matmuls (folded with the next Dense layer). Data-parallel over 8 cores.

v2: fp16 datapath. x is cast fp32->fp16 by a gpsimd DMA into DRAM scratch,
then DMA-transposed (XBAR) straight into SBUF in feature-major layout -- no
PE transposes or PSUM->SBUF copies on the input side. All matmuls are fp16
(1 cyc/row vs 4 for fp32). DVE/ACT passes are 1024 wide; partition shuffles
run on a uint32 view (halves 1x-mode shuffle cost).
"""
import sys
sys.path.insert(0, "/opt/trn_rl_repo")
import itertools
import contextlib
import numpy as np

import concourse.bass as bass
import concourse.bacc as bacc
import concourse.tile as tile
from concourse import mybir
from concourse.bass_utils import run_bass_kernel_spmd
from concourse.masks import make_identity

F32 = mybir.dt.float32
F16 = mybir.dt.float16
F32R = mybir.dt.float32r
U32 = mybir.dt.uint32
PI2 = float(np.pi / 2)
N_CORES = 8
B_TOTAL, D_IN = 524288, 16
B_CORE = B_TOTAL // N_CORES
N_SS = B_CORE // 16384  # supersteps of 16384 rows
CH = 4                  # chunks of 4096 rows per superstep

# ---------------- host-side math ----------------
_I2 = np.eye(2, dtype=complex)
_PY = np.array([[0, -1j], [1j, 0]])
_PZ = np.array([[1, 0], [0, -1]], dtype=complex)
SUPPORTS = [(0, 1, 3), (0, 2, 3), (1, 3), (0, 2)]


def _kron(ms):
    out = np.array([[1.0 + 0j]])
    for m in ms:
        out = np.kron(out, m)
    return out


def _op_on(w, m):
    return _kron([m if v == w else _I2 for v in range(4)])


def _layer_tensors(theta_l):
    U = np.eye(16, dtype=complex)
    for l in range(2):
        for w in range(4):
            c, s = np.cos(theta_l[l, w] / 2), np.sin(theta_l[l, w] / 2)
            U = _op_on(w, np.array([[c, -1j * s], [-1j * s, c]])) @ U
        for w in range(4):
            t = (w + 1) % 4
            C = np.zeros((16, 16), dtype=complex)
            for k in range(16):
                bits = [(k >> (3 - v)) & 1 for v in range(4)]
                if bits[w] == 1:
                    bits[t] ^= 1
                C[sum(b << (3 - v) for v, b in enumerate(bits)), k] = 1
            U = C @ U
    letters = {"I": _I2, "Y": _PY, "Z": _PZ}
    out = []
    for w, sup in enumerate(SUPPORTS):
        H = U.conj().T @ _op_on(w, _PZ) @ U
        T = np.zeros((2,) * len(sup))
        for s in itertools.product("IYZ", repeat=4):
            P = _kron([letters[c] for c in s])
            co = float(np.real(np.trace(P.conj().T @ H) / 16))
            if abs(co) < 1e-10:
                continue
            nz = tuple(v for v in range(4) if s[v] != "I")
            assert set(nz).issubset(set(sup)), f"support {s} w={w}"
            idx, sign = [], 1.0
            ok = True
            for v in sup:
                if s[v] == "I":
                    ok = False
                    break
                idx.append(0 if s[v] == "Y" else 1)
                if s[v] == "Y":
                    sign = -sign
            if not ok:
                assert abs(co) < 1e-10
                continue
            T[tuple(idx)] = sign * co
        out.append(T)
    return out  # C0, C1, B2, B3


def _blockdiag(blk, n):
    K, M = blk.shape
    out = np.zeros((K * n, M * n), dtype=np.float32)
    for i in range(n):
        out[i * K:(i + 1) * K, i * M:(i + 1) * M] = blk
    return out


WMAP = [3, 0, 1, 2, 3, 0, 1, 2]


def _to_bf16(a):
    return a.astype(np.float16)


def host_tensors(theta, W0, b0, W1, b1, W2, b2):
    t = {}
    for i in range(3):
        C0, C1, B2, B3 = _layer_tensors(np.asarray(theta[i], dtype=np.float64))
        A1 = np.zeros((8, 8), dtype=np.float32)
        for a in range(2):
            for c in range(2):
                gi = a * 2 + c
                A1[gi, 1] = C0[a, 0, c]
                A1[gi, 5] = C0[a, 1, c]
                A1[gi, 2] = C1[a, 0, c]
                A1[gi, 6] = C1[a, 1, c]
        A2 = np.zeros((8, 8), dtype=np.float32)
        for b in range(2):
            A2[1 + 4 * b, 3] = B2[b, 0]
            A2[1 + 4 * b, 7] = B2[b, 1]
            A2[2 + 4 * b, 0] = B3[0, b]
            A2[2 + 4 * b, 4] = B3[1, b]
        t[f"lA1_{i}"] = _blockdiag(A1, 16)
        t[f"lA2_{i}"] = _blockdiag(A2, 16)
    D0 = np.zeros((16, 8), dtype=np.float32)
    D0[:, 0:4] = W0
    D0[:, 4:8] = W0
    t["lD0"] = _blockdiag(D0, 8)
    for i, W in [(1, W1), (2, W2)]:
        D = np.zeros((8, 8), dtype=np.float32)
        for k in range(8):
            for j in range(4):
                D[k, j] = W[WMAP[k], j]
                D[k, j + 4] = W[WMAP[k], j]
        t[f"lD{i}"] = _blockdiag(D, 16)
    PO = np.zeros((8, 4), dtype=np.float32)
    for k in range(8):
        PO[k, WMAP[k]] = 1.0
    t["lPO"] = _blockdiag(PO, 16)
    consts = np.zeros((128, 4), dtype=np.float32)
    for i, b in enumerate((b0, b1, b2)):
        consts[:, i] = np.tile(np.tile(np.asarray(b, np.float32), 2), 16)
    consts[:, 3] = np.tile([0., 0., 0., 0., PI2, PI2, PI2, PI2], 16)
    t["consts"] = consts
    for name in list(t):
        if name != "consts":
            t[name] = _to_bf16(t[name])
    return t


# ---------------- device kernel ----------------
MASK_A = [0, 0, 4, 4, 0, 0, 0, 0]
MASK_B = [3, 7, 3, 7, 0, 0, 0, 0]
W16_NAMES = ["lD0", "lD1", "lD2", "lA1_0", "lA2_0", "lA1_1", "lA2_1",
             "lA1_2", "lA2_2", "lPO"]
W_NAMES = W16_NAMES + ["consts"]
W_COLS = {"lD0": 64, "lD1": 128, "lD2": 128, "lA1_0": 128, "lA2_0": 128,
          "lA1_1": 128, "lA2_1": 128, "lA1_2": 128, "lA2_2": 128,
          "lPO": 64, "consts": 4}


N_ST = 8  # steps of 8192 rows
N_STAGES = 11


def build_kernel(tc, x, out, wins):
    nc = tc.nc
    shufA = [8 * t_ + MASK_A[j] for t_ in range(4) for j in range(8)]
    shufB = [8 * t_ + MASK_B[j] for t_ in range(4) for j in range(8)]
    with contextlib.ExitStack() as ctx:
        wpool = ctx.enter_context(tc.tile_pool(name="w", bufs=1))
        dram = ctx.enter_context(tc.tile_pool(name="dram", bufs=8, space="DRAM"))
        slabp = ctx.enter_context(tc.tile_pool(name="slab", bufs=8))
        sxkp = ctx.enter_context(tc.tile_pool(name="sxk", bufs=3))
        hp = ctx.enter_context(tc.tile_pool(name="hp", bufs=3))
        tp = ctx.enter_context(tc.tile_pool(name="tp", bufs=4))
        gp = ctx.enter_context(tc.tile_pool(name="gp", bufs=3))
        vp = ctx.enter_context(tc.tile_pool(name="vp", bufs=5))
        outp = ctx.enter_context(tc.tile_pool(name="outp", bufs=3))
        ps_xk = ctx.enter_context(tc.tile_pool(name="ps_xk", bufs=2, space="PSUM"))
        ps_pre = ctx.enter_context(tc.tile_pool(name="ps_pre", bufs=3, space="PSUM"))
        ps_r1 = ctx.enter_context(tc.tile_pool(name="ps_r1", bufs=2, space="PSUM"))
        ps_ob = ctx.enter_context(tc.tile_pool(name="ps_ob", bufs=1, space="PSUM"))

        wt = {}
        for name in W16_NAMES:
            wtile = wpool.tile([128, W_COLS[name]], F16, tag=name)
            nc.sync.dma_start(wtile[:], wins[name][:, :])
            wt[name] = wtile
        ctile = wpool.tile([128, 4], F32, tag="consts")
        nc.sync.dma_start(ctile[:], wins["consts"][:, :])
        identf = wpool.tile([128, 128], F16, tag="identf")
        make_identity(nc, identf)
        lA1 = [wt["lA1_0"], wt["lA1_1"], wt["lA1_2"]]
        lA2 = [wt["lA2_0"], wt["lA2_1"], wt["lA2_2"]]
        lD = [None, wt["lD1"], wt["lD2"]]

        xin = x.rearrange("(st k) f -> st k f", st=N_ST)
        ov = out.rearrange("(st s p r) w -> st s p (r w)", st=N_ST, s=4, p=128)

        # upfront fp32->fp16 casts into DRAM scratch (SWDGE, big descriptors)
        x16s = []
        for st in range(N_ST):
            x16 = dram.tile([8192, 16], F16, tag="x16", name="x16")
            nc.gpsimd.dma_start(x16[:], xin[st])
            x16s.append(x16)

        sxks = [None] * N_ST
        pres = [None] * N_ST
        trigs = [None] * N_ST
        gs = [None] * N_ST
        r1s = [None] * N_ST
        vins = [None] * N_ST

        def s_ingest(st):
            x16v = x16s[st][:].rearrange("(s p r) f -> s p (r f)", s=4, p=128)
            xkps = ps_xk.tile([128, 1024], F16, tag="xkps", name="xkps")
            for s in range(4):
                slab = slabp.tile([128, 256], F16, tag="slab", name="slab")
                nc.sync.dma_start(slab[:], x16v[s])
                nc.tensor.transpose(xkps[:, s * 128:s * 128 + 128],
                                    slab[:, 0:128], identf[:])
                nc.tensor.transpose(xkps[:, 512 + s * 128:512 + s * 128 + 128],
                                    slab[:, 128:256], identf[:])
            sxk = sxkp.tile([128, 1024], F16, tag="sxk", name="sxk")
            nc.vector.tensor_copy(sxk[:, 0:512], xkps[:, 0:512])
            nc.vector.tensor_copy(sxk[:, 512:1024], xkps[:, 512:1024])
            sxks[st] = sxk

        def s_act(st, li):
            pre = ps_pre.tile([128, 512], F32, tag="pre", name="pre")
            if li == 0:
                nc.tensor.matmul(pre[0:64, :], wt["lD0"][:],
                                 sxks[st][:, 0:512], start=True, stop=True)
                nc.tensor.matmul(pre[64:128, :], wt["lD0"][:],
                                 sxks[st][:, 512:1024], start=True, stop=True)
                sxks[st] = None
            else:
                nc.tensor.matmul(pre[:], lD[li][:], vins[st][:],
                                 start=True, stop=True)
            h8 = hp.tile([128, 512], F16, tag="h8", name="h8")
            nc.scalar.activation(h8[:], pre[:],
                                 mybir.ActivationFunctionType.Tanh,
                                 bias=ctile[:, li:li + 1], scale=1.0)
            trig = tp.tile([128, 512], F16, tag="trig", name="trig")
            nc.scalar.activation(trig[:], h8[:],
                                 mybir.ActivationFunctionType.Sin,
                                 bias=ctile[:, 3:4], scale=1.0)
            trigs[st] = trig

        def s_shuf(st, li):
            ga = gp.tile([128, 512], F16, tag="ga", name="ga")
            gb = gp.tile([128, 512], F16, tag="gb", name="gb")
            nc.vector.stream_shuffle(ga[:].bitcast(U32),
                                     trigs[st][:].bitcast(U32), shufA)
            nc.vector.stream_shuffle(gb[:].bitcast(U32),
                                     trigs[st][:].bitcast(U32), shufB)
            g = gp.tile([128, 512], F16, tag="g", name="g")
            nc.gpsimd.tensor_mul(g[:], ga[:], gb[:])
            gs[st] = g

        def s_av(st, li):
            r1 = ps_r1.tile([128, 512], F32, tag="r1", name="r1")
            nc.tensor.matmul(r1[:], lA1[li][:], gs[st][:], start=True, stop=False)
            nc.tensor.matmul(r1[:], lA2[li][:], trigs[st][:], start=False, stop=True)
            v = vp.tile([128, 512], F16, tag="v", name="v")
            nc.vector.tensor_mul(v[:], trigs[st][:], r1[:])
            vins[st] = v

        def s_tail(st):
            ob = ps_ob.tile([128, 256], F32, tag="ob", name="ob")
            for s in range(4):
                nc.tensor.matmul(ob[:, s * 64:(s + 1) * 64],
                                 vins[st][:, s * 128:(s + 1) * 128], wt["lPO"][:],
                                 start=True, stop=True)
            sob = outp.tile([128, 256], F32, tag="sob", name="sob")
            nc.scalar.copy(sob[:], ob[:])
            for s in range(4):
                nc.sync.dma_start(ov[st, s], sob[:, s * 64:(s + 1) * 64])

        def emit_stage(st, k):
            if k == 0:
                s_ingest(st)
            elif k <= 9:
                li, sub = (k - 1) // 3, (k - 1) % 3
                (s_act, s_shuf, s_av)[sub](st, li)
            else:
                s_tail(st)

        # diagonal wave emission: stage k of step st at wave st + k, so every
        # engine's in-order stream interleaves all steps at staggered stages
        for w in range(N_STAGES + N_ST):
            for st in range(min(st_max := N_ST, w + 1)):
                k = w - st
                if 0 <= k < N_STAGES:
                    emit_stage(st, k)


# Force Tanh/Sin into a single resident ACT table set (silu_and_others holds
# both) so the table-load pass doesn't thrash between per-func sets. Dict
# order/indices are preserved so act_func_set_id stays consistent.
from concourse import hw_specs as _hw_specs
import concourse.bacc as _bacc_mod
_orig_get_tables = _hw_specs.get_activation_tables

def _patched_get_tables(arch):
    tabs = _orig_get_tables(arch)
    out = {}
    for name, s in tabs.items():
        s2 = set(s)
        if name != "silu_and_others":
            s2.discard(mybir.ActivationFunctionType.Tanh)
            s2.discard(mybir.ActivationFunctionType.Sin)
        out[name] = s2
    return out

_hw_specs.get_activation_tables = _patched_get_tables
for _mod in (_bacc_mod,):
    if hasattr(_mod, "get_activation_tables"):
        _mod.get_activation_tables = _patched_get_tables


_CACHE = {}


def _get_compiled():
    if "nc" in _CACHE:
        return _CACHE["nc"], _CACHE["tiles"]
    nc = bacc.Bacc("TRN2", target_bir_lowering=False, debug=False,
                   num_devices=N_CORES)
    x_ap = nc.dram_tensor("x", [B_CORE, D_IN], F32, kind="ExternalInput").ap()
    out_ap = nc.dram_tensor("out", [B_CORE, 4], F32, kind="ExternalOutput").ap()
    wins = {}
    for name in W16_NAMES:
        wins[name] = nc.dram_tensor(name, [128, W_COLS[name]], F16,
                                    kind="ExternalInput").ap()
    wins["consts"] = nc.dram_tensor("consts", [128, 4], F32,
                                    kind="ExternalInput").ap()
    with tile.TileContext(nc) as tc:
        build_kernel(tc, x_ap, out_ap, wins)
    nc.compile()
    _CACHE["nc"] = nc
    _CACHE["tiles"] = None
    return nc, None


def kernel(x, theta, W0, b0, W1, b1, W2, b2):
    x = np.ascontiguousarray(np.asarray(x, dtype=np.float32))
    wt = host_tensors(np.asarray(theta), np.asarray(W0), np.asarray(b0),
                      np.asarray(W1), np.asarray(b1), np.asarray(W2),
                      np.asarray(b2))
    nc, _ = _get_compiled()
    in_maps = []
    for c in range(N_CORES):
        m = {"x": np.ascontiguousarray(x[c * B_CORE:(c + 1) * B_CORE])}
        for name in W_NAMES:
            m[name] = wt[name]
        in_maps.append(m)
    res = run_bass_kernel_spmd(nc, in_maps, core_ids=list(range(N_CORES)))
    outs = [res.results[c]["out"] for c in range(N_CORES)]
    return np.concatenate(outs, axis=0).astype(np.float32)


# revision 25
# speedup vs baseline: 2.2442x; 1.0654x over previous
"""Self-contained Trainium2 Bass kernel for the HQNN problem (v2).

Math: the 4-qubit circuit after angle embedding applies a fixed unitary whose
Heisenberg-evolved Z observables are sparse Pauli sums over {I,Y,Z}; each
hybrid layer reduces to tanh -> sin/cos -> a few elementwise products -> small
static matmuls (folded with the next Dense layer). Data-parallel over 8 cores.

v2: fp16 datapath. x is cast fp32->fp16 by a gpsimd DMA into DRAM scratch,
then DMA-transposed (XBAR) straight into SBUF in feature-major layout -- no
PE transposes or PSUM->SBUF copies on the input side. All matmuls are fp16
(1 cyc/row vs 4 for fp32). DVE/ACT passes are 1024 wide; partition shuffles
run on a uint32 view (halves 1x-mode shuffle cost).
"""
import sys
sys.path.insert(0, "/opt/trn_rl_repo")
import itertools
import contextlib
import numpy as np

import concourse.bass as bass
import concourse.bacc as bacc
import concourse.tile as tile
from concourse import mybir
from concourse.bass_utils import run_bass_kernel_spmd
from concourse.masks import make_identity

F32 = mybir.dt.float32
F16 = mybir.dt.float16
F32R = mybir.dt.float32r
U32 = mybir.dt.uint32
PI2 = float(np.pi / 2)
N_CORES = 8
B_TOTAL, D_IN = 524288, 16
B_CORE = B_TOTAL // N_CORES
N_SS = B_CORE // 16384  # supersteps of 16384 rows
CH = 4                  # chunks of 4096 rows per superstep

# ---------------- host-side math ----------------
_I2 = np.eye(2, dtype=complex)
_PY = np.array([[0, -1j], [1j, 0]])
_PZ = np.array([[1, 0], [0, -1]], dtype=complex)
SUPPORTS = [(0, 1, 3), (0, 2, 3), (1, 3), (0, 2)]


def _kron(ms):
    out = np.array([[1.0 + 0j]])
    for m in ms:
        out = np.kron(out, m)
    return out


def _op_on(w, m):
    return _kron([m if v == w else _I2 for v in range(4)])


def _layer_tensors(theta_l):
    U = np.eye(16, dtype=complex)
    for l in range(2):
        for w in range(4):
            c, s = np.cos(theta_l[l, w] / 2), np.sin(theta_l[l, w] / 2)
            U = _op_on(w, np.array([[c, -1j * s], [-1j * s, c]])) @ U
        for w in range(4):
            t = (w + 1) % 4
            C = np.zeros((16, 16), dtype=complex)
            for k in range(16):
                bits = [(k >> (3 - v)) & 1 for v in range(4)]
                if bits[w] == 1:
                    bits[t] ^= 1
                C[sum(b << (3 - v) for v, b in enumerate(bits)), k] = 1
            U = C @ U
    letters = {"I": _I2, "Y": _PY, "Z": _PZ}
    out = []
    for w, sup in enumerate(SUPPORTS):
        H = U.conj().T @ _op_on(w, _PZ) @ U
        T = np.zeros((2,) * len(sup))
        for s in itertools.product("IYZ", repeat=4):
            P = _kron([letters[c] for c in s])
            co = float(np.real(np.trace(P.conj().T @ H) / 16))
            if abs(co) < 1e-10:
                continue
            nz = tuple(v for v in range(4) if s[v] != "I")
            assert set(nz).issubset(set(sup)), f"support {s} w={w}"
            idx, sign = [], 1.0
            ok = True
            for v in sup:
                if s[v] == "I":
                    ok = False
                    break
                idx.append(0 if s[v] == "Y" else 1)
                if s[v] == "Y":
                    sign = -sign
            if not ok:
                assert abs(co) < 1e-10
                continue
            T[tuple(idx)] = sign * co
        out.append(T)
    return out  # C0, C1, B2, B3


def _blockdiag(blk, n):
    K, M = blk.shape
    out = np.zeros((K * n, M * n), dtype=np.float32)
    for i in range(n):
        out[i * K:(i + 1) * K, i * M:(i + 1) * M] = blk
    return out


WMAP = [3, 0, 1, 2, 3, 0, 1, 2]


def _to_bf16(a):
    return a.astype(np.float16)


def host_tensors(theta, W0, b0, W1, b1, W2, b2):
    t = {}
    for i in range(3):
        C0, C1, B2, B3 = _layer_tensors(np.asarray(theta[i], dtype=np.float64))
        A1 = np.zeros((8, 8), dtype=np.float32)
        for a in range(2):
            for c in range(2):
                gi = a * 2 + c
                A1[gi, 1] = C0[a, 0, c]
                A1[gi, 5] = C0[a, 1, c]
                A1[gi, 2] = C1[a, 0, c]
                A1[gi, 6] = C1[a, 1, c]
        A2 = np.zeros((8, 8), dtype=np.float32)
        for b in range(2):
            A2[1 + 4 * b, 3] = B2[b, 0]
            A2[1 + 4 * b, 7] = B2[b, 1]
            A2[2 + 4 * b, 0] = B3[0, b]
            A2[2 + 4 * b, 4] = B3[1, b]
        t[f"lA1_{i}"] = _blockdiag(A1, 16)
        t[f"lA2_{i}"] = _blockdiag(A2, 16)
    D0 = np.zeros((16, 8), dtype=np.float32)
    D0[:, 0:4] = W0
    D0[:, 4:8] = W0
    t["lD0"] = _blockdiag(D0, 8)
    for i, W in [(1, W1), (2, W2)]:
        D = np.zeros((8, 8), dtype=np.float32)
        for k in range(8):
            for j in range(4):
                D[k, j] = W[WMAP[k], j]
                D[k, j + 4] = W[WMAP[k], j]
        t[f"lD{i}"] = _blockdiag(D, 16)
    PO = np.zeros((8, 4), dtype=np.float32)
    for k in range(8):
        PO[k, WMAP[k]] = 1.0
    t["lPO"] = _blockdiag(PO, 16)
    consts = np.zeros((128, 4), dtype=np.float32)
    for i, b in enumerate((b0, b1, b2)):
        consts[:, i] = np.tile(np.tile(np.asarray(b, np.float32), 2), 16)
    consts[:, 3] = np.tile([0., 0., 0., 0., PI2, PI2, PI2, PI2], 16)
    t["consts"] = consts
    for name in list(t):
        if name != "consts":
            t[name] = _to_bf16(t[name])
    t["wpack"] = np.concatenate([t[n] for n in W16_NAMES], axis=1)
    return t


# ---------------- device kernel ----------------
MASK_A = [0, 0, 4, 4, 0, 0, 0, 0]
MASK_B = [3, 7, 3, 7, 0, 0, 0, 0]
W16_NAMES = ["lD0", "lD1", "lD2", "lA1_0", "lA2_0", "lA1_1", "lA2_1",
             "lA1_2", "lA2_2", "lPO"]
W_NAMES = W16_NAMES + ["consts"]
W_COLS = {"lD0": 64, "lD1": 128, "lD2": 128, "lA1_0": 128, "lA2_0": 128,
          "lA1_1": 128, "lA2_1": 128, "lA1_2": 128, "lA2_2": 128,
          "lPO": 64, "consts": 4}


N_ST = 8  # steps of 8192 rows
N_STAGES = 11


def build_kernel(tc, x, out, wins):
    nc = tc.nc
    shufA = [8 * t_ + MASK_A[j] for t_ in range(4) for j in range(8)]
    shufB = [8 * t_ + MASK_B[j] for t_ in range(4) for j in range(8)]
    with contextlib.ExitStack() as ctx:
        wpool = ctx.enter_context(tc.tile_pool(name="w", bufs=1))
        dram = ctx.enter_context(tc.tile_pool(name="dram", bufs=8, space="DRAM"))
        slabp = ctx.enter_context(tc.tile_pool(name="slab", bufs=3))
        sxkp = ctx.enter_context(tc.tile_pool(name="sxk", bufs=3))
        hp = ctx.enter_context(tc.tile_pool(name="hp", bufs=3))
        tp = ctx.enter_context(tc.tile_pool(name="tp", bufs=4))
        gp = ctx.enter_context(tc.tile_pool(name="gp", bufs=3))
        vp = ctx.enter_context(tc.tile_pool(name="vp", bufs=5))
        outp = ctx.enter_context(tc.tile_pool(name="outp", bufs=3))
        ps_xk = ctx.enter_context(tc.tile_pool(name="ps_xk", bufs=2, space="PSUM"))
        ps_pre = ctx.enter_context(tc.tile_pool(name="ps_pre", bufs=3, space="PSUM"))
        ps_r1 = ctx.enter_context(tc.tile_pool(name="ps_r1", bufs=2, space="PSUM"))
        ps_ob = ctx.enter_context(tc.tile_pool(name="ps_ob", bufs=1, space="PSUM"))

        wpack = wpool.tile([128, 1152], F16, tag="wpack")
        nc.sync.dma_start(wpack[:], wins["wpack"][:, :])
        wt = {}
        off = 0
        for name in W16_NAMES:
            wt[name] = wpack[:, off:off + W_COLS[name]]
            off += W_COLS[name]
        ctile = wpool.tile([128, 4], F32, tag="consts")
        nc.sync.dma_start(ctile[:], wins["consts"][:, :])
        identf = wpool.tile([128, 128], F16, tag="identf")
        make_identity(nc, identf)
        lA1 = [wt["lA1_0"], wt["lA1_1"], wt["lA1_2"]]
        lA2 = [wt["lA2_0"], wt["lA2_1"], wt["lA2_2"]]
        lD = [None, wt["lD1"], wt["lD2"]]

        xin = x.rearrange("(st k) f -> st k f", st=N_ST)
        ov = out.rearrange("(st s p r) w -> st p s (r w)", st=N_ST, s=4, p=128)

        # upfront fp32->fp16 casts into DRAM scratch (SWDGE, big descriptors)
        x16s = []
        for st in range(N_ST):
            x16 = dram.tile([8192, 16], F16, tag="x16", name="x16")
            nc.gpsimd.dma_start(x16[:], xin[st])
            x16s.append(x16)

        sxks = [None] * N_ST
        pres = [None] * N_ST
        trigs = [None] * N_ST
        gs = [None] * N_ST
        r1s = [None] * N_ST
        vins = [None] * N_ST

        def s_ingest(st):
            x16v = x16s[st][:].rearrange("(s p r) f -> p s (r f)", s=4, p=128)
            xkps = ps_xk.tile([128, 1024], F16, tag="xkps", name="xkps")
            slab = slabp.tile([128, 4, 256], F16, tag="slab", name="slab")
            nc.sync.dma_start(slab[:], x16v)
            for s in range(4):
                nc.tensor.transpose(xkps[:, s * 128:s * 128 + 128],
                                    slab[:, s, 0:128], identf[:])
                nc.tensor.transpose(xkps[:, 512 + s * 128:512 + s * 128 + 128],
                                    slab[:, s, 128:256], identf[:])
            sxk = sxkp.tile([128, 1024], F16, tag="sxk", name="sxk")
            nc.vector.tensor_copy(sxk[:, 0:512], xkps[:, 0:512])
            nc.vector.tensor_copy(sxk[:, 512:1024], xkps[:, 512:1024])
            sxks[st] = sxk

        def s_act(st, li):
            pre = ps_pre.tile([128, 512], F32, tag="pre", name="pre")
            if li == 0:
                nc.tensor.matmul(pre[0:64, :], wt["lD0"],
                                 sxks[st][:, 0:512], start=True, stop=True)
                nc.tensor.matmul(pre[64:128, :], wt["lD0"],
                                 sxks[st][:, 512:1024], start=True, stop=True)
                sxks[st] = None
            else:
                nc.tensor.matmul(pre[:], lD[li], vins[st][:],
                                 start=True, stop=True)
            h8 = hp.tile([128, 512], F16, tag="h8", name="h8")
            nc.scalar.activation(h8[:], pre[:],
                                 mybir.ActivationFunctionType.Tanh,
                                 bias=ctile[:, li:li + 1], scale=1.0)
            trig = tp.tile([128, 512], F16, tag="trig", name="trig")
            nc.scalar.activation(trig[:], h8[:],
                                 mybir.ActivationFunctionType.Sin,
                                 bias=ctile[:, 3:4], scale=1.0)
            trigs[st] = trig

        def s_shuf(st, li):
            ga = gp.tile([128, 512], F16, tag="ga", name="ga")
            gb = gp.tile([128, 512], F16, tag="gb", name="gb")
            nc.vector.stream_shuffle(ga[:].bitcast(U32),
                                     trigs[st][:].bitcast(U32), shufA)
            nc.vector.stream_shuffle(gb[:].bitcast(U32),
                                     trigs[st][:].bitcast(U32), shufB)
            g = gp.tile([128, 512], F16, tag="g", name="g")
            nc.gpsimd.tensor_mul(g[:], ga[:], gb[:])
            gs[st] = g

        def s_av(st, li):
            r1 = ps_r1.tile([128, 512], F32, tag="r1", name="r1")
            nc.tensor.matmul(r1[:], lA1[li], gs[st][:], start=True, stop=False)
            nc.tensor.matmul(r1[:], lA2[li], trigs[st][:], start=False, stop=True)
            v = vp.tile([128, 512], F16, tag="v", name="v")
            nc.vector.tensor_mul(v[:], trigs[st][:], r1[:])
            vins[st] = v

        def s_tail(st):
            ob = ps_ob.tile([128, 256], F32, tag="ob", name="ob")
            for s in range(4):
                nc.tensor.matmul(ob[:, s * 64:(s + 1) * 64],
                                 vins[st][:, s * 128:(s + 1) * 128], wt["lPO"],
                                 start=True, stop=True)
            sob = outp.tile([128, 256], F32, tag="sob", name="sob")
            nc.scalar.copy(sob[:], ob[:])
            nc.sync.dma_start(ov[st], sob[:])

        def emit_stage(st, k):
            if k == 0:
                s_ingest(st)
            elif k <= 9:
                li, sub = (k - 1) // 3, (k - 1) % 3
                (s_act, s_shuf, s_av)[sub](st, li)
            else:
                s_tail(st)

        # diagonal wave emission: stage k of step st at wave st + k, so every
        # engine's in-order stream interleaves all steps at staggered stages
        for w in range(N_STAGES + N_ST):
            for st in range(min(st_max := N_ST, w + 1)):
                k = w - st
                if 0 <= k < N_STAGES:
                    emit_stage(st, k)


# Force Tanh/Sin into a single resident ACT table set (silu_and_others holds
# both) so the table-load pass doesn't thrash between per-func sets. Dict
# order/indices are preserved so act_func_set_id stays consistent.
from concourse import hw_specs as _hw_specs
import concourse.bacc as _bacc_mod
_orig_get_tables = _hw_specs.get_activation_tables

def _patched_get_tables(arch):
    tabs = _orig_get_tables(arch)
    out = {}
    for name, s in tabs.items():
        s2 = set(s)
        if name != "silu_and_others":
            s2.discard(mybir.ActivationFunctionType.Tanh)
            s2.discard(mybir.ActivationFunctionType.Sin)
        out[name] = s2
    return out

_hw_specs.get_activation_tables = _patched_get_tables
for _mod in (_bacc_mod,):
    if hasattr(_mod, "get_activation_tables"):
        _mod.get_activation_tables = _patched_get_tables


_CACHE = {}


def _get_compiled():
    if "nc" in _CACHE:
        return _CACHE["nc"], _CACHE["tiles"]
    nc = bacc.Bacc("TRN2", target_bir_lowering=False, debug=False,
                   num_devices=N_CORES)
    x_ap = nc.dram_tensor("x", [B_CORE, D_IN], F32, kind="ExternalInput").ap()
    out_ap = nc.dram_tensor("out", [B_CORE, 4], F32, kind="ExternalOutput").ap()
    wins = {}
    wins["wpack"] = nc.dram_tensor("wpack", [128, 1152], F16,
                                   kind="ExternalInput").ap()
    wins["consts"] = nc.dram_tensor("consts", [128, 4], F32,
                                    kind="ExternalInput").ap()
    with tile.TileContext(nc) as tc:
        build_kernel(tc, x_ap, out_ap, wins)
    nc.compile()
    _CACHE["nc"] = nc
    _CACHE["tiles"] = None
    return nc, None


def kernel(x, theta, W0, b0, W1, b1, W2, b2):
    x = np.ascontiguousarray(np.asarray(x, dtype=np.float32))
    wt = host_tensors(np.asarray(theta), np.asarray(W0), np.asarray(b0),
                      np.asarray(W1), np.asarray(b1), np.asarray(W2),
                      np.asarray(b2))
    nc, _ = _get_compiled()
    in_maps = []
    for c in range(N_CORES):
        m = {"x": np.ascontiguousarray(x[c * B_CORE:(c + 1) * B_CORE]),
             "wpack": wt["wpack"], "consts": wt["consts"]}
        in_maps.append(m)
    res = run_bass_kernel_spmd(nc, in_maps, core_ids=list(range(N_CORES)))
    outs = [res.results[c]["out"] for c in range(N_CORES)]
    return np.concatenate(outs, axis=0).astype(np.float32)


# revision 27
# speedup vs baseline: 2.2707x; 1.0118x over previous
"""Self-contained Trainium2 Bass kernel for the HQNN problem (v2).

Math: the 4-qubit circuit after angle embedding applies a fixed unitary whose
Heisenberg-evolved Z observables are sparse Pauli sums over {I,Y,Z}; each
hybrid layer reduces to tanh -> sin/cos -> a few elementwise products -> small
static matmuls (folded with the next Dense layer). Data-parallel over 8 cores.

v2: fp16 datapath. x is cast fp32->fp16 by a gpsimd DMA into DRAM scratch,
then DMA-transposed (XBAR) straight into SBUF in feature-major layout -- no
PE transposes or PSUM->SBUF copies on the input side. All matmuls are fp16
(1 cyc/row vs 4 for fp32). DVE/ACT passes are 1024 wide; partition shuffles
run on a uint32 view (halves 1x-mode shuffle cost).
"""
import sys
sys.path.insert(0, "/opt/trn_rl_repo")
import itertools
import contextlib
import numpy as np

import concourse.bass as bass
import concourse.bacc as bacc
import concourse.tile as tile
from concourse import mybir
from concourse.bass_utils import run_bass_kernel_spmd
from concourse.masks import make_identity

F32 = mybir.dt.float32
F16 = mybir.dt.float16
F32R = mybir.dt.float32r
U32 = mybir.dt.uint32
PI2 = float(np.pi / 2)
N_CORES = 8
B_TOTAL, D_IN = 524288, 16
B_CORE = B_TOTAL // N_CORES
N_SS = B_CORE // 16384  # supersteps of 16384 rows
CH = 4                  # chunks of 4096 rows per superstep

# ---------------- host-side math ----------------
_I2 = np.eye(2, dtype=complex)
_PY = np.array([[0, -1j], [1j, 0]])
_PZ = np.array([[1, 0], [0, -1]], dtype=complex)
SUPPORTS = [(0, 1, 3), (0, 2, 3), (1, 3), (0, 2)]


def _kron(ms):
    out = np.array([[1.0 + 0j]])
    for m in ms:
        out = np.kron(out, m)
    return out


def _op_on(w, m):
    return _kron([m if v == w else _I2 for v in range(4)])


def _layer_tensors(theta_l):
    U = np.eye(16, dtype=complex)
    for l in range(2):
        for w in range(4):
            c, s = np.cos(theta_l[l, w] / 2), np.sin(theta_l[l, w] / 2)
            U = _op_on(w, np.array([[c, -1j * s], [-1j * s, c]])) @ U
        for w in range(4):
            t = (w + 1) % 4
            C = np.zeros((16, 16), dtype=complex)
            for k in range(16):
                bits = [(k >> (3 - v)) & 1 for v in range(4)]
                if bits[w] == 1:
                    bits[t] ^= 1
                C[sum(b << (3 - v) for v, b in enumerate(bits)), k] = 1
            U = C @ U
    letters = {"I": _I2, "Y": _PY, "Z": _PZ}
    out = []
    for w, sup in enumerate(SUPPORTS):
        H = U.conj().T @ _op_on(w, _PZ) @ U
        T = np.zeros((2,) * len(sup))
        for s in itertools.product("IYZ", repeat=4):
            P = _kron([letters[c] for c in s])
            co = float(np.real(np.trace(P.conj().T @ H) / 16))
            if abs(co) < 1e-10:
                continue
            nz = tuple(v for v in range(4) if s[v] != "I")
            assert set(nz).issubset(set(sup)), f"support {s} w={w}"
            idx, sign = [], 1.0
            ok = True
            for v in sup:
                if s[v] == "I":
                    ok = False
                    break
                idx.append(0 if s[v] == "Y" else 1)
                if s[v] == "Y":
                    sign = -sign
            if not ok:
                assert abs(co) < 1e-10
                continue
            T[tuple(idx)] = sign * co
        out.append(T)
    return out  # C0, C1, B2, B3


def _blockdiag(blk, n):
    K, M = blk.shape
    out = np.zeros((K * n, M * n), dtype=np.float32)
    for i in range(n):
        out[i * K:(i + 1) * K, i * M:(i + 1) * M] = blk
    return out


WMAP = [3, 0, 1, 2, 3, 0, 1, 2]


def _to_bf16(a):
    return a.astype(np.float16)


def host_tensors(theta, W0, b0, W1, b1, W2, b2):
    t = {}
    for i in range(3):
        C0, C1, B2, B3 = _layer_tensors(np.asarray(theta[i], dtype=np.float64))
        A1 = np.zeros((8, 8), dtype=np.float32)
        for a in range(2):
            for c in range(2):
                gi = a * 2 + c
                A1[gi, 1] = C0[a, 0, c]
                A1[gi, 5] = C0[a, 1, c]
                A1[gi, 2] = C1[a, 0, c]
                A1[gi, 6] = C1[a, 1, c]
        A2 = np.zeros((8, 8), dtype=np.float32)
        for b in range(2):
            A2[1 + 4 * b, 3] = B2[b, 0]
            A2[1 + 4 * b, 7] = B2[b, 1]
            A2[2 + 4 * b, 0] = B3[0, b]
            A2[2 + 4 * b, 4] = B3[1, b]
        t[f"lA1_{i}"] = _blockdiag(A1, 16)
        t[f"lA2_{i}"] = _blockdiag(A2, 16)
    D0 = np.zeros((16, 8), dtype=np.float32)
    D0[:, 0:4] = W0
    D0[:, 4:8] = W0
    t["lD0"] = _blockdiag(D0, 8)
    for i, W in [(1, W1), (2, W2)]:
        D = np.zeros((8, 8), dtype=np.float32)
        for k in range(8):
            for j in range(4):
                D[k, j] = W[WMAP[k], j]
                D[k, j + 4] = W[WMAP[k], j]
        t[f"lD{i}"] = _blockdiag(D, 16)
    PO = np.zeros((8, 4), dtype=np.float32)
    for k in range(8):
        PO[k, WMAP[k]] = 1.0
    t["lPO"] = _blockdiag(PO, 16)
    consts = np.zeros((128, 4), dtype=np.float32)
    for i, b in enumerate((b0, b1, b2)):
        consts[:, i] = np.tile(np.tile(np.asarray(b, np.float32), 2), 16)
    consts[:, 3] = np.tile([0., 0., 0., 0., PI2, PI2, PI2, PI2], 16)
    t["consts"] = consts
    for name in list(t):
        if name != "consts":
            t[name] = _to_bf16(t[name])
    t["wpack"] = np.concatenate([t[n] for n in W16_NAMES], axis=1)
    return t


# ---------------- device kernel ----------------
MASK_A = [0, 0, 4, 4, 0, 0, 0, 0]
MASK_B = [3, 7, 3, 7, 0, 0, 0, 0]
W16_NAMES = ["lD0", "lD1", "lD2", "lA1_0", "lA2_0", "lA1_1", "lA2_1",
             "lA1_2", "lA2_2", "lPO"]
W_NAMES = W16_NAMES + ["consts"]
W_COLS = {"lD0": 64, "lD1": 128, "lD2": 128, "lA1_0": 128, "lA2_0": 128,
          "lA1_1": 128, "lA2_1": 128, "lA1_2": 128, "lA2_2": 128,
          "lPO": 64, "consts": 4}


N_ST = 8  # steps of 8192 rows
N_STAGES = 11


def build_kernel(tc, x, out, wins):
    nc = tc.nc
    shufA = [8 * t_ + MASK_A[j] for t_ in range(4) for j in range(8)]
    shufB = [8 * t_ + MASK_B[j] for t_ in range(4) for j in range(8)]
    with contextlib.ExitStack() as ctx:
        wpool = ctx.enter_context(tc.tile_pool(name="w", bufs=1))
        dram = ctx.enter_context(tc.tile_pool(name="dram", bufs=8, space="DRAM"))
        slabp = ctx.enter_context(tc.tile_pool(name="slab", bufs=3))
        sxkp = ctx.enter_context(tc.tile_pool(name="sxk", bufs=3))
        hp = ctx.enter_context(tc.tile_pool(name="hp", bufs=3))
        tp = ctx.enter_context(tc.tile_pool(name="tp", bufs=4))
        gp = ctx.enter_context(tc.tile_pool(name="gp", bufs=3))
        vp = ctx.enter_context(tc.tile_pool(name="vp", bufs=5))
        outp = ctx.enter_context(tc.tile_pool(name="outp", bufs=3))
        ps_xk = ctx.enter_context(tc.tile_pool(name="ps_xk", bufs=2, space="PSUM"))
        ps_pre = ctx.enter_context(tc.tile_pool(name="ps_pre", bufs=2, space="PSUM"))
        ps_r1 = ctx.enter_context(tc.tile_pool(name="ps_r1", bufs=2, space="PSUM"))
        ps_ob = ctx.enter_context(tc.tile_pool(name="ps_ob", bufs=1, space="PSUM"))

        wpack = wpool.tile([128, 1152], F16, tag="wpack")
        nc.sync.dma_start(wpack[:], wins["wpack"][:, :])
        wt = {}
        off = 0
        for name in W16_NAMES:
            wt[name] = wpack[:, off:off + W_COLS[name]]
            off += W_COLS[name]
        ctile = wpool.tile([128, 4], F32, tag="consts")
        nc.sync.dma_start(ctile[:], wins["consts"][:, :])
        identf = wpool.tile([128, 128], F16, tag="identf")
        make_identity(nc, identf)
        ident32 = wpool.tile([128, 128], F32, tag="ident32")
        make_identity(nc, ident32)
        lA1 = [wt["lA1_0"], wt["lA1_1"], wt["lA1_2"]]
        lA2 = [wt["lA2_0"], wt["lA2_1"], wt["lA2_2"]]
        lD = [None, wt["lD1"], wt["lD2"]]

        xin = x.rearrange("(st k) f -> st k f", st=N_ST)
        xv32 = x.rearrange("(st s p r) f -> st p s (r f)", st=N_ST, s=4, p=128)
        ov = out.rearrange("(st s p r) w -> st p s (r w)", st=N_ST, s=4, p=128)

        # upfront fp32->fp16 casts into DRAM scratch (SWDGE, big descriptors);
        # steps 0-1 ingest directly from fp32 x so nothing waits on the casts
        x16s = [None] * N_ST
        for st in range(2, N_ST):
            x16 = dram.tile([8192, 16], F16, tag="x16", name="x16")
            nc.gpsimd.dma_start(x16[:], xin[st])
            x16s[st] = x16

        sxks = [None] * N_ST
        pres = [None] * N_ST
        trigs = [None] * N_ST
        gs = [None] * N_ST
        r1s = [None] * N_ST
        vins = [None] * N_ST

        def s_ingest(st):
            sxk = sxkp.tile([128, 1024], F16, tag="sxk", name="sxk")
            if st < 2:
                slab = slabp.tile([128, 4, 256], F32, tag="slab32", name="slab")
                nc.sync.dma_start(slab[:], xv32[st])
                for hh in range(2):
                    xk32 = ps_xk.tile([128, 512], F32, tag="xk32", name="xk32")
                    for s in range(4):
                        nc.tensor.transpose(xk32[:, s * 128:s * 128 + 128],
                                            slab[:, s, hh * 128:hh * 128 + 128],
                                            ident32[:])
                    nc.vector.tensor_copy(sxk[:, hh * 512:(hh + 1) * 512], xk32[:])
            else:
                x16v = x16s[st][:].rearrange("(s p r) f -> p s (r f)", s=4, p=128)
                xkps = ps_xk.tile([128, 1024], F16, tag="xkps", name="xkps", bufs=1)
                slab = slabp.tile([128, 4, 256], F16, tag="slab", name="slab")
                nc.sync.dma_start(slab[:], x16v)
                for s in range(4):
                    nc.tensor.transpose(xkps[:, s * 128:s * 128 + 128],
                                        slab[:, s, 0:128], identf[:])
                    nc.tensor.transpose(xkps[:, 512 + s * 128:512 + s * 128 + 128],
                                        slab[:, s, 128:256], identf[:])
                nc.vector.tensor_copy(sxk[:, 0:512], xkps[:, 0:512])
                nc.vector.tensor_copy(sxk[:, 512:1024], xkps[:, 512:1024])
            sxks[st] = sxk

        def s_act(st, li):
            pre = ps_pre.tile([128, 512], F32, tag="pre", name="pre")
            if li == 0:
                nc.tensor.matmul(pre[0:64, :], wt["lD0"],
                                 sxks[st][:, 0:512], start=True, stop=True)
                nc.tensor.matmul(pre[64:128, :], wt["lD0"],
                                 sxks[st][:, 512:1024], start=True, stop=True)
                sxks[st] = None
            else:
                nc.tensor.matmul(pre[:], lD[li], vins[st][:],
                                 start=True, stop=True)
            h8 = hp.tile([128, 512], F16, tag="h8", name="h8")
            nc.scalar.activation(h8[:], pre[:],
                                 mybir.ActivationFunctionType.Tanh,
                                 bias=ctile[:, li:li + 1], scale=1.0)
            trig = tp.tile([128, 512], F16, tag="trig", name="trig")
            nc.scalar.activation(trig[:], h8[:],
                                 mybir.ActivationFunctionType.Sin,
                                 bias=ctile[:, 3:4], scale=1.0)
            trigs[st] = trig

        def s_shuf(st, li):
            ga = gp.tile([128, 512], F16, tag="ga", name="ga")
            gb = gp.tile([128, 512], F16, tag="gb", name="gb")
            nc.vector.stream_shuffle(ga[:].bitcast(U32),
                                     trigs[st][:].bitcast(U32), shufA)
            nc.vector.stream_shuffle(gb[:].bitcast(U32),
                                     trigs[st][:].bitcast(U32), shufB)
            g = gp.tile([128, 512], F16, tag="g", name="g")
            nc.gpsimd.tensor_mul(g[:], ga[:], gb[:])
            gs[st] = g

        def s_av(st, li):
            r1 = ps_r1.tile([128, 512], F32, tag="r1", name="r1")
            nc.tensor.matmul(r1[:], lA1[li], gs[st][:], start=True, stop=False)
            nc.tensor.matmul(r1[:], lA2[li], trigs[st][:], start=False, stop=True)
            v = vp.tile([128, 512], F16, tag="v", name="v")
            nc.vector.tensor_mul(v[:], trigs[st][:], r1[:])
            vins[st] = v

        def s_tail(st):
            ob = ps_ob.tile([128, 256], F32, tag="ob", name="ob")
            for s in range(4):
                nc.tensor.matmul(ob[:, s * 64:(s + 1) * 64],
                                 vins[st][:, s * 128:(s + 1) * 128], wt["lPO"],
                                 start=True, stop=True)
            sob = outp.tile([128, 256], F32, tag="sob", name="sob")
            nc.scalar.copy(sob[:], ob[:])
            nc.sync.dma_start(ov[st], sob[:])

        def emit_stage(st, k):
            if k == 0:
                s_ingest(st)
            elif k <= 9:
                li, sub = (k - 1) // 3, (k - 1) % 3
                (s_act, s_shuf, s_av)[sub](st, li)
            else:
                s_tail(st)

        # diagonal wave emission: stage k of step st at wave st + k, so every
        # engine's in-order stream interleaves all steps at staggered stages
        for w in range(N_STAGES + N_ST):
            for st in range(min(st_max := N_ST, w + 1)):
                k = w - st
                if 0 <= k < N_STAGES:
                    emit_stage(st, k)


# Force Tanh/Sin into a single resident ACT table set (silu_and_others holds
# both) so the table-load pass doesn't thrash between per-func sets. Dict
# order/indices are preserved so act_func_set_id stays consistent.
from concourse import hw_specs as _hw_specs
import concourse.bacc as _bacc_mod
_orig_get_tables = _hw_specs.get_activation_tables

def _patched_get_tables(arch):
    tabs = _orig_get_tables(arch)
    out = {}
    for name, s in tabs.items():
        s2 = set(s)
        if name != "silu_and_others":
            s2.discard(mybir.ActivationFunctionType.Tanh)
            s2.discard(mybir.ActivationFunctionType.Sin)
        out[name] = s2
    return out

_hw_specs.get_activation_tables = _patched_get_tables
for _mod in (_bacc_mod,):
    if hasattr(_mod, "get_activation_tables"):
        _mod.get_activation_tables = _patched_get_tables


_CACHE = {}


def _get_compiled():
    if "nc" in _CACHE:
        return _CACHE["nc"], _CACHE["tiles"]
    nc = bacc.Bacc("TRN2", target_bir_lowering=False, debug=False,
                   num_devices=N_CORES)
    x_ap = nc.dram_tensor("x", [B_CORE, D_IN], F32, kind="ExternalInput").ap()
    out_ap = nc.dram_tensor("out", [B_CORE, 4], F32, kind="ExternalOutput").ap()
    wins = {}
    wins["wpack"] = nc.dram_tensor("wpack", [128, 1152], F16,
                                   kind="ExternalInput").ap()
    wins["consts"] = nc.dram_tensor("consts", [128, 4], F32,
                                    kind="ExternalInput").ap()
    with tile.TileContext(nc) as tc:
        build_kernel(tc, x_ap, out_ap, wins)
    nc.compile()
    _CACHE["nc"] = nc
    _CACHE["tiles"] = None
    return nc, None


def kernel(x, theta, W0, b0, W1, b1, W2, b2):
    x = np.ascontiguousarray(np.asarray(x, dtype=np.float32))
    wt = host_tensors(np.asarray(theta), np.asarray(W0), np.asarray(b0),
                      np.asarray(W1), np.asarray(b1), np.asarray(W2),
                      np.asarray(b2))
    nc, _ = _get_compiled()
    in_maps = []
    for c in range(N_CORES):
        m = {"x": np.ascontiguousarray(x[c * B_CORE:(c + 1) * B_CORE]),
             "wpack": wt["wpack"], "consts": wt["consts"]}
        in_maps.append(m)
    res = run_bass_kernel_spmd(nc, in_maps, core_ids=list(range(N_CORES)))
    outs = [res.results[c]["out"] for c in range(N_CORES)]
    return np.concatenate(outs, axis=0).astype(np.float32)


# revision 28
# speedup vs baseline: 2.3106x; 1.0175x over previous
"""Self-contained Trainium2 Bass kernel for the HQNN problem (v2).

Math: the 4-qubit circuit after angle embedding applies a fixed unitary whose
Heisenberg-evolved Z observables are sparse Pauli sums over {I,Y,Z}; each
hybrid layer reduces to tanh -> sin/cos -> a few elementwise products -> small
static matmuls (folded with the next Dense layer). Data-parallel over 8 cores.

v2: fp16 datapath. x is cast fp32->fp16 by a gpsimd DMA into DRAM scratch,
then DMA-transposed (XBAR) straight into SBUF in feature-major layout -- no
PE transposes or PSUM->SBUF copies on the input side. All matmuls are fp16
(1 cyc/row vs 4 for fp32). DVE/ACT passes are 1024 wide; partition shuffles
run on a uint32 view (halves 1x-mode shuffle cost).
"""
import sys
sys.path.insert(0, "/opt/trn_rl_repo")
import itertools
import contextlib
import numpy as np

import concourse.bass as bass
import concourse.bacc as bacc
import concourse.tile as tile
from concourse import mybir
from concourse.bass_utils import run_bass_kernel_spmd
from concourse.masks import make_identity

F32 = mybir.dt.float32
F16 = mybir.dt.float16
F32R = mybir.dt.float32r
U32 = mybir.dt.uint32
PI2 = float(np.pi / 2)
N_CORES = 8
B_TOTAL, D_IN = 524288, 16
B_CORE = B_TOTAL // N_CORES
N_SS = B_CORE // 16384  # supersteps of 16384 rows
CH = 4                  # chunks of 4096 rows per superstep

# ---------------- host-side math ----------------
_I2 = np.eye(2, dtype=complex)
_PY = np.array([[0, -1j], [1j, 0]])
_PZ = np.array([[1, 0], [0, -1]], dtype=complex)
SUPPORTS = [(0, 1, 3), (0, 2, 3), (1, 3), (0, 2)]


def _kron(ms):
    out = np.array([[1.0 + 0j]])
    for m in ms:
        out = np.kron(out, m)
    return out


def _op_on(w, m):
    return _kron([m if v == w else _I2 for v in range(4)])


def _layer_tensors(theta_l):
    U = np.eye(16, dtype=complex)
    for l in range(2):
        for w in range(4):
            c, s = np.cos(theta_l[l, w] / 2), np.sin(theta_l[l, w] / 2)
            U = _op_on(w, np.array([[c, -1j * s], [-1j * s, c]])) @ U
        for w in range(4):
            t = (w + 1) % 4
            C = np.zeros((16, 16), dtype=complex)
            for k in range(16):
                bits = [(k >> (3 - v)) & 1 for v in range(4)]
                if bits[w] == 1:
                    bits[t] ^= 1
                C[sum(b << (3 - v) for v, b in enumerate(bits)), k] = 1
            U = C @ U
    letters = {"I": _I2, "Y": _PY, "Z": _PZ}
    out = []
    for w, sup in enumerate(SUPPORTS):
        H = U.conj().T @ _op_on(w, _PZ) @ U
        T = np.zeros((2,) * len(sup))
        for s in itertools.product("IYZ", repeat=4):
            P = _kron([letters[c] for c in s])
            co = float(np.real(np.trace(P.conj().T @ H) / 16))
            if abs(co) < 1e-10:
                continue
            nz = tuple(v for v in range(4) if s[v] != "I")
            assert set(nz).issubset(set(sup)), f"support {s} w={w}"
            idx, sign = [], 1.0
            ok = True
            for v in sup:
                if s[v] == "I":
                    ok = False
                    break
                idx.append(0 if s[v] == "Y" else 1)
                if s[v] == "Y":
                    sign = -sign
            if not ok:
                assert abs(co) < 1e-10
                continue
            T[tuple(idx)] = sign * co
        out.append(T)
    return out  # C0, C1, B2, B3


def _blockdiag(blk, n):
    K, M = blk.shape
    out = np.zeros((K * n, M * n), dtype=np.float32)
    for i in range(n):
        out[i * K:(i + 1) * K, i * M:(i + 1) * M] = blk
    return out


WMAP = [3, 0, 1, 2, 3, 0, 1, 2]


def _to_bf16(a):
    return a.astype(np.float16)


def host_tensors(theta, W0, b0, W1, b1, W2, b2):
    t = {}
    for i in range(3):
        C0, C1, B2, B3 = _layer_tensors(np.asarray(theta[i], dtype=np.float64))
        A1 = np.zeros((8, 8), dtype=np.float32)
        for a in range(2):
            for c in range(2):
                gi = a * 2 + c
                A1[gi, 1] = C0[a, 0, c]
                A1[gi, 5] = C0[a, 1, c]
                A1[gi, 2] = C1[a, 0, c]
                A1[gi, 6] = C1[a, 1, c]
        A2 = np.zeros((8, 8), dtype=np.float32)
        for b in range(2):
            A2[1 + 4 * b, 3] = B2[b, 0]
            A2[1 + 4 * b, 7] = B2[b, 1]
            A2[2 + 4 * b, 0] = B3[0, b]
            A2[2 + 4 * b, 4] = B3[1, b]
        t[f"lA1_{i}"] = _blockdiag(A1, 16)
        t[f"lA2_{i}"] = _blockdiag(A2, 16)
    D0 = np.zeros((16, 8), dtype=np.float32)
    D0[:, 0:4] = W0
    D0[:, 4:8] = W0
    t["lD0"] = _blockdiag(D0, 8)
    for i, W in [(1, W1), (2, W2)]:
        D = np.zeros((8, 8), dtype=np.float32)
        for k in range(8):
            for j in range(4):
                D[k, j] = W[WMAP[k], j]
                D[k, j + 4] = W[WMAP[k], j]
        t[f"lD{i}"] = _blockdiag(D, 16)
    PO = np.zeros((8, 4), dtype=np.float32)
    for k in range(8):
        PO[k, WMAP[k]] = 1.0
    t["lPO"] = _blockdiag(PO, 16)
    consts = np.zeros((128, 4), dtype=np.float32)
    for i, b in enumerate((b0, b1, b2)):
        consts[:, i] = np.tile(np.tile(np.asarray(b, np.float32), 2), 16)
    consts[:, 3] = np.tile([0., 0., 0., 0., PI2, PI2, PI2, PI2], 16)
    t["consts"] = consts
    for name in list(t):
        if name != "consts":
            t[name] = _to_bf16(t[name])
    t["wpack"] = np.concatenate([t[n] for n in W16_NAMES], axis=1)
    return t


# ---------------- device kernel ----------------
MASK_A = [0, 0, 4, 4, 0, 0, 0, 0]
MASK_B = [3, 7, 3, 7, 0, 0, 0, 0]
W16_NAMES = ["lD0", "lD1", "lD2", "lA1_0", "lA2_0", "lA1_1", "lA2_1",
             "lA1_2", "lA2_2", "lPO"]
W_NAMES = W16_NAMES + ["consts"]
W_COLS = {"lD0": 64, "lD1": 128, "lD2": 128, "lA1_0": 128, "lA2_0": 128,
          "lA1_1": 128, "lA2_1": 128, "lA1_2": 128, "lA2_2": 128,
          "lPO": 64, "consts": 4}


N_ST = 8  # steps of 8192 rows
N_STAGES = 11


def build_kernel(tc, x, out, wins):
    nc = tc.nc
    shufA = [8 * t_ + MASK_A[j] for t_ in range(4) for j in range(8)]
    shufB = [8 * t_ + MASK_B[j] for t_ in range(4) for j in range(8)]
    with contextlib.ExitStack() as ctx:
        wpool = ctx.enter_context(tc.tile_pool(name="w", bufs=1))
        dram = ctx.enter_context(tc.tile_pool(name="dram", bufs=8, space="DRAM"))
        slabp = ctx.enter_context(tc.tile_pool(name="slab", bufs=3))
        sxkp = ctx.enter_context(tc.tile_pool(name="sxk", bufs=3))
        hp = ctx.enter_context(tc.tile_pool(name="hp", bufs=3))
        tp = ctx.enter_context(tc.tile_pool(name="tp", bufs=4))
        gp = ctx.enter_context(tc.tile_pool(name="gp", bufs=3))
        vp = ctx.enter_context(tc.tile_pool(name="vp", bufs=5))
        outp = ctx.enter_context(tc.tile_pool(name="outp", bufs=3))
        ps_xk = ctx.enter_context(tc.tile_pool(name="ps_xk", bufs=2, space="PSUM"))
        ps_pre = ctx.enter_context(tc.tile_pool(name="ps_pre", bufs=3, space="PSUM"))
        ps_r1 = ctx.enter_context(tc.tile_pool(name="ps_r1", bufs=2, space="PSUM"))
        ps_ob = ctx.enter_context(tc.tile_pool(name="ps_ob", bufs=1, space="PSUM"))

        wpack = wpool.tile([128, 1152], F16, tag="wpack")
        nc.sync.dma_start(wpack[:], wins["wpack"][:, :])
        wt = {}
        off = 0
        for name in W16_NAMES:
            wt[name] = wpack[:, off:off + W_COLS[name]]
            off += W_COLS[name]
        ctile = wpool.tile([128, 4], F32, tag="consts")
        nc.sync.dma_start(ctile[:], wins["consts"][:, :])
        identf = wpool.tile([128, 128], F16, tag="identf")
        make_identity(nc, identf)
        ident32 = wpool.tile([128, 128], F32, tag="ident32")
        make_identity(nc, ident32)
        lA1 = [wt["lA1_0"], wt["lA1_1"], wt["lA1_2"]]
        lA2 = [wt["lA2_0"], wt["lA2_1"], wt["lA2_2"]]
        lD = [None, wt["lD1"], wt["lD2"]]

        xin = x.rearrange("(st k) f -> st k f", st=N_ST)
        xv32 = x.rearrange("(st s p r) f -> st p s (r f)", st=N_ST, s=4, p=128)
        ov = out.rearrange("(st s p r) w -> st p s (r w)", st=N_ST, s=4, p=128)

        # fp32->fp16 casts into DRAM scratch (SWDGE); issued just-in-time in
        # the wave loop so their slow cast-ALU packets don't starve the
        # early slab loads. Steps 0-1 ingest directly from fp32 x.
        x16s = [None] * N_ST

        def s_cast(st):
            x16 = dram.tile([8192, 16], F16, tag="x16", name="x16")
            nc.gpsimd.dma_start(x16[:], xin[st])
            x16s[st] = x16

        sxks = [None] * N_ST
        pres = [None] * N_ST
        trigs = [None] * N_ST
        gs = [None] * N_ST
        r1s = [None] * N_ST
        vins = [None] * N_ST

        def s_ingest(st):
            sxk = sxkp.tile([128, 1024], F16, tag="sxk", name="sxk")
            if st < 2:
                slab = slabp.tile([128, 4, 256], F32, tag="slab32", name="slab")
                nc.sync.dma_start(slab[:], xv32[st])
                for hh in range(2):
                    xk32 = ps_xk.tile([128, 512], F32, tag="xk32", name="xk32", bufs=1)
                    for s in range(4):
                        nc.tensor.transpose(xk32[:, s * 128:s * 128 + 128],
                                            slab[:, s, hh * 128:hh * 128 + 128],
                                            ident32[:])
                    nc.vector.tensor_copy(sxk[:, hh * 512:(hh + 1) * 512], xk32[:])
            else:
                x16v = x16s[st][:].rearrange("(s p r) f -> p s (r f)", s=4, p=128)
                xkps = ps_xk.tile([128, 1024], F16, tag="xkps", name="xkps", bufs=1)
                slab = slabp.tile([128, 4, 256], F16, tag="slab", name="slab")
                nc.sync.dma_start(slab[:], x16v)
                for s in range(4):
                    nc.tensor.transpose(xkps[:, s * 128:s * 128 + 128],
                                        slab[:, s, 0:128], identf[:])
                    nc.tensor.transpose(xkps[:, 512 + s * 128:512 + s * 128 + 128],
                                        slab[:, s, 128:256], identf[:])
                nc.vector.tensor_copy(sxk[:, 0:512], xkps[:, 0:512])
                nc.vector.tensor_copy(sxk[:, 512:1024], xkps[:, 512:1024])
            sxks[st] = sxk

        def s_act(st, li):
            pre = ps_pre.tile([128, 512], F32, tag="pre", name="pre")
            if li == 0:
                nc.tensor.matmul(pre[0:64, :], wt["lD0"],
                                 sxks[st][:, 0:512], start=True, stop=True)
                nc.tensor.matmul(pre[64:128, :], wt["lD0"],
                                 sxks[st][:, 512:1024], start=True, stop=True)
                sxks[st] = None
            else:
                nc.tensor.matmul(pre[:], lD[li], vins[st][:],
                                 start=True, stop=True)
            h8 = hp.tile([128, 512], F16, tag="h8", name="h8")
            nc.scalar.activation(h8[:], pre[:],
                                 mybir.ActivationFunctionType.Tanh,
                                 bias=ctile[:, li:li + 1], scale=1.0)
            trig = tp.tile([128, 512], F16, tag="trig", name="trig")
            nc.scalar.activation(trig[:], h8[:],
                                 mybir.ActivationFunctionType.Sin,
                                 bias=ctile[:, 3:4], scale=1.0)
            trigs[st] = trig

        def s_shuf(st, li):
            ga = gp.tile([128, 512], F16, tag="ga", name="ga")
            gb = gp.tile([128, 512], F16, tag="gb", name="gb")
            nc.vector.stream_shuffle(ga[:].bitcast(U32),
                                     trigs[st][:].bitcast(U32), shufA)
            nc.vector.stream_shuffle(gb[:].bitcast(U32),
                                     trigs[st][:].bitcast(U32), shufB)
            g = gp.tile([128, 512], F16, tag="g", name="g")
            nc.gpsimd.tensor_mul(g[:], ga[:], gb[:])
            gs[st] = g

        def s_av(st, li):
            r1 = ps_r1.tile([128, 512], F32, tag="r1", name="r1")
            nc.tensor.matmul(r1[:], lA1[li], gs[st][:], start=True, stop=False)
            nc.tensor.matmul(r1[:], lA2[li], trigs[st][:], start=False, stop=True)
            v = vp.tile([128, 512], F16, tag="v", name="v")
            nc.vector.tensor_mul(v[:], trigs[st][:], r1[:])
            vins[st] = v

        def s_tail(st):
            ob = ps_ob.tile([128, 256], F32, tag="ob", name="ob")
            for s in range(4):
                nc.tensor.matmul(ob[:, s * 64:(s + 1) * 64],
                                 vins[st][:, s * 128:(s + 1) * 128], wt["lPO"],
                                 start=True, stop=True)
            sob = outp.tile([128, 256], F32, tag="sob", name="sob")
            nc.scalar.copy(sob[:], ob[:])
            nc.sync.dma_start(ov[st], sob[:])

        def emit_stage(st, k):
            if k == 0:
                s_ingest(st)
            elif k <= 9:
                li, sub = (k - 1) // 3, (k - 1) % 3
                (s_act, s_shuf, s_av)[sub](st, li)
            else:
                s_tail(st)

        # diagonal wave emission: stage k of step st at wave st + k, so every
        # engine's in-order stream interleaves all steps at staggered stages
        for w in range(N_STAGES + N_ST):
            stc = w + 2
            if 2 <= stc < N_ST:
                s_cast(stc)
            for st in range(min(N_ST, w + 1)):
                k = w - st
                if 0 <= k < N_STAGES:
                    emit_stage(st, k)


# Force Tanh/Sin into a single resident ACT table set (silu_and_others holds
# both) so the table-load pass doesn't thrash between per-func sets. Dict
# order/indices are preserved so act_func_set_id stays consistent.
from concourse import hw_specs as _hw_specs
import concourse.bacc as _bacc_mod
_orig_get_tables = _hw_specs.get_activation_tables

def _patched_get_tables(arch):
    tabs = _orig_get_tables(arch)
    out = {}
    for name, s in tabs.items():
        s2 = set(s)
        if name != "silu_and_others":
            s2.discard(mybir.ActivationFunctionType.Tanh)
            s2.discard(mybir.ActivationFunctionType.Sin)
        out[name] = s2
    return out

_hw_specs.get_activation_tables = _patched_get_tables
for _mod in (_bacc_mod,):
    if hasattr(_mod, "get_activation_tables"):
        _mod.get_activation_tables = _patched_get_tables


_CACHE = {}


def _get_compiled():
    if "nc" in _CACHE:
        return _CACHE["nc"], _CACHE["tiles"]
    nc = bacc.Bacc("TRN2", target_bir_lowering=False, debug=False,
                   num_devices=N_CORES)
    x_ap = nc.dram_tensor("x", [B_CORE, D_IN], F32, kind="ExternalInput").ap()
    out_ap = nc.dram_tensor("out", [B_CORE, 4], F32, kind="ExternalOutput").ap()
    wins = {}
    wins["wpack"] = nc.dram_tensor("wpack", [128, 1152], F16,
                                   kind="ExternalInput").ap()
    wins["consts"] = nc.dram_tensor("consts", [128, 4], F32,
                                    kind="ExternalInput").ap()
    with tile.TileContext(nc) as tc:
        build_kernel(tc, x_ap, out_ap, wins)
    nc.compile()
    _CACHE["nc"] = nc
    _CACHE["tiles"] = None
    return nc, None


def kernel(x, theta, W0, b0, W1, b1, W2, b2):
    x = np.ascontiguousarray(np.asarray(x, dtype=np.float32))
    wt = host_tensors(np.asarray(theta), np.asarray(W0), np.asarray(b0),
                      np.asarray(W1), np.asarray(b1), np.asarray(W2),
                      np.asarray(b2))
    nc, _ = _get_compiled()
    in_maps = []
    for c in range(N_CORES):
        m = {"x": np.ascontiguousarray(x[c * B_CORE:(c + 1) * B_CORE]),
             "wpack": wt["wpack"], "consts": wt["consts"]}
        in_maps.append(m)
    res = run_bass_kernel_spmd(nc, in_maps, core_ids=list(range(N_CORES)))
    outs = [res.results[c]["out"] for c in range(N_CORES)]
    return np.concatenate(outs, axis=0).astype(np.float32)
